# revision 20
# baseline (speedup 1.0000x reference)
"""Trainium2 Bass kernel for nn_BiT_Phoneme (dense transformer).

Data-parallel: 16 batch elems / 8 cores = 2 per core; each core runs the
full network on its 2 sequences (1024 "tokens"). Activations are kept
feature-major ([dim on partitions, tokens on free]) so matmuls chain
without transposes. LayerNorm stats use ones-vector matmuls (partition
reduction). Softmax runs transposed (keys on partitions) as
exp(scores)*E with E = exp(rel_bias)*causal_mask precomputed on host;
denominators come from a ones column appended to V; normalization is a
reciprocal broadcast. Matmuls use float32r (full PE rate, ~1.5e-4);
attention probs / V / h1 / W2 use bf16.
"""

import numpy as np

import concourse.bass as bass
import concourse.mybir as mybir
import concourse.tile as tile
from concourse import bacc
from concourse.bass_utils import run_bass_kernel_spmd

B, T, F = 16, 2048, 256
PH = 4
PATCH = 1024
DIM = 1024
DEPTH = 6
HEADS, DHEAD = 16, 64
INNER = 1024
MLP = 4096
NCLS = 41
MAXREL = 200
KSIZE, SIGMA = 20, 2.0
EPS = 1e-5
SEQ = T // PH              # 512
NCORES = 8
BPC = B // NCORES          # 2
TOK = BPC * SEQ            # 1024
P = 128

DT_R = mybir.dt.float32r
DT_F = mybir.dt.float32
DT_H = mybir.dt.bfloat16
FX = mybir.ActivationFunctionType
OP = mybir.AluOpType

DTILES = DIM // P          # 8
KTILES = DIM // P          # 8
MTILES = MLP // P          # 32
SEQT = SEQ // P            # 4


def build_nc():
    nc = bacc.Bacc(None, target_bir_lowering=False)

    par = {}
    def dp(name, shape, dtype, is_out=False):
        par[name] = nc.declare_dram_parameter(name, list(shape), dtype, isOutput=is_out)
        return par[name]

    dp("xin", (BPC, T, F), DT_R)
    dp("band", (T // P, 3, P, P), DT_R)
    dp("etab", (DEPTH, SEQT, P, SEQ), DT_F)
    dp("wpe", (PATCH, DIM), DT_R)
    dp("wqkv", (DEPTH, DIM, 3 * INNER), DT_R)
    dp("wo", (DEPTH, INNER, DIM), DT_R)
    dp("w1", (DEPTH, DIM, MLP), DT_R)
    dp("w2", (DEPTH, MLP, DIM), DT_H)
    dp("wproj", (DIM, NCLS), DT_R)
    for nm, shp in [("lnp1g", (PATCH,)), ("lnp1b", (PATCH,)), ("bpe", (DIM,)),
                    ("lnp2g", (DIM,)), ("lnp2b", (DIM,)),
                    ("lnag", (DEPTH, DIM)), ("lnab", (DEPTH, DIM)),
                    ("lnfg", (DEPTH, DIM)), ("lnfb", (DEPTH, DIM)),
                    ("bov", (DEPTH, DIM)), ("b1v", (DEPTH, MLP)),
                    ("b2v", (DEPTH, DIM)), ("lnog", (DIM,)), ("lnob", (DIM,)),
                    ("bprojv", (NCLS,))]:
        dp(nm, shp, DT_F)
    dp("out", (NCLS, TOK), DT_F, is_out=True)

    with tile.TileContext(nc) as tc:
        _emit(nc, tc, par)
    nc.compile()
    return nc


def _emit(nc, tc, par):
    import contextlib
    ctx = contextlib.ExitStack()
    with ctx:
        const = ctx.enter_context(tc.tile_pool(name="const", bufs=1))
        xpool = ctx.enter_context(tc.tile_pool(name="xpool", bufs=1))
        big = ctx.enter_context(tc.tile_pool(name="big", bufs=2))
        wsm = ctx.enter_context(tc.tile_pool(name="wsm", bufs=3))
        stats = ctx.enter_context(tc.tile_pool(name="stats", bufs=1))
        sm2 = ctx.enter_context(tc.tile_pool(name="sm2", bufs=2))
        one1 = ctx.enter_context(tc.tile_pool(name="one1", bufs=1))
        psm = ctx.enter_context(tc.tile_pool(name="psm", bufs=4, space="PSUM"))
        pso = ctx.enter_context(tc.tile_pool(name="pso", bufs=2, space="PSUM"))
        pst = ctx.enter_context(tc.tile_pool(name="pst", bufs=1, space="PSUM"))

        ones_r = const.tile([P, 1], DT_R, name="ones_r")
        nc.vector.memset(ones_r.bitcast(mybir.dt.uint32), 0x3F800000)
        ones_h = const.tile([P, 1], DT_H, name="ones_h")
        nc.vector.memset(ones_h.bitcast(mybir.dt.uint16), 0x3F80)
        epst = const.tile([1, 1], DT_F, name="epst")
        nc.vector.memset(epst, EPS)

        def load_vec(nm, width):
            d = par[nm]
            if len(d.shape) == 1:
                tl = const.tile([P, width // P], DT_F, name=nm + "_t")
                nc.sync.dma_start(out=tl, in_=d.rearrange("(o p) -> p o", p=P))
            else:
                L = d.shape[0]
                tl = const.tile([P, L, width // P], DT_F, name=nm + "_t")
                nc.sync.dma_start(out=tl, in_=d.rearrange("l (o p) -> p l o", p=P))
            return tl

        lnp1g_t = load_vec("lnp1g", PATCH)
        lnp1b_t = load_vec("lnp1b", PATCH)
        bpe_t = load_vec("bpe", DIM)
        lnp2g_t = load_vec("lnp2g", DIM)
        lnp2b_t = load_vec("lnp2b", DIM)
        lnag_t = load_vec("lnag", DIM)
        lnab_t = load_vec("lnab", DIM)
        lnfg_t = load_vec("lnfg", DIM)
        lnfb_t = load_vec("lnfb", DIM)
        bov_t = load_vec("bov", DIM)
        b1v_t = load_vec("b1v", MLP)
        b2v_t = load_vec("b2v", DIM)
        lnog_t = load_vec("lnog", DIM)
        lnob_t = load_vec("lnob", DIM)
        bproj_t = const.tile([NCLS, 1], DT_F, name="bproj_t")
        nc.sync.dma_start(out=bproj_t,
                          in_=par["bprojv"].rearrange("(p o) -> p o", o=1))

        x = xpool.tile([P, DTILES, TOK], DT_R, name="x")

        # ---- layernorm over partitions+tiles (feature-major) ----
        # views(d) -> [P, width] fp32r source slices (ntiles of them);
        # writes dst_fn(d) slices, width tokens, nh 512-halves.
        def layer_norm_fm(views, dst_fn, ntiles, D, width, g_fn, b_fn):
            nh = width // 512
            st = stats.tile([1, 2, TOK], DT_F, name="st")
            mu, rstd = st[:, 0, :width], st[:, 1, :width]
            for th in range(nh):
                sl = bass.ts(th, 512)
                ps0 = pst.tile([1, 512], DT_F, name="ps0")
                ps1 = pst.tile([1, 512], DT_F, name="ps1")
                for d in range(ntiles):
                    v = views(d)[:, sl]
                    sq = sm2.tile([P, 512], DT_R, name="sq")
                    nc.vector.tensor_mul(sq, v, v)
                    nc.tensor.matmul(ps0, ones_r, v,
                                     start=(d == 0), stop=(d == ntiles - 1))
                    nc.tensor.matmul(ps1, ones_r, sq,
                                     start=(d == 0), stop=(d == ntiles - 1))
                nc.scalar.mul(mu[:, sl], ps0, 1.0 / D)
                nc.scalar.mul(ps1, ps1, 1.0 / D)
                # rstd slot <- mu^2, then var = E[x^2] - mu^2
                nc.scalar.activation(rstd[:, sl], mu[:, sl], FX.Square)
                nc.vector.tensor_sub(rstd[:, sl], ps1, rstd[:, sl])
            nc.scalar.activation(rstd, rstd, FX.Sqrt, bias=epst, scale=1.0)
            nc.vector.reciprocal(rstd, rstd)
            mrb = stats.tile([P, 2, TOK], DT_F, name="mrb")
            nc.gpsimd.partition_broadcast(mrb[:, 0, :width], mu)
            nc.gpsimd.partition_broadcast(mrb[:, 1, :width], rstd)
            for d in range(ntiles):
                for th in range(nh):
                    sl = bass.ts(th, 512)
                    tmp = sm2.tile([P, 512], DT_F, name="lntmp")
                    nc.vector.tensor_sub(tmp, views(d)[:, sl], mrb[:, 0, sl])
                    nc.vector.tensor_mul(tmp, tmp, mrb[:, 1, sl])
                    nc.vector.tensor_scalar(
                        dst_fn(d)[:, sl], tmp, g_fn(d), b_fn(d),
                        OP.mult, OP.add)

        # =================== embedding ===================
        xin, band = par["xin"], par["band"]
        with (
            tc.tile_pool(name="sfp", bufs=1) as sfp,
            tc.tile_pool(name="pnp", bufs=1) as pnp,
        ):
            x0 = big.tile([P, DTILES, TOK], DT_R, name="bigbuf")
            for b in range(BPC):
                sf = sfp.tile([P, 2, T], DT_R, name="sf")
                for fh in range(2):
                    for g4 in range(T // 512):
                        pg = psm.tile([P, 512], DT_F, name="pmain")
                        for q in range(4):
                            ct = g4 * 4 + q
                            bt = wsm.tile([P, 3, P], DT_R, name="wsm_t")
                            nc.sync.dma_start(
                                out=bt,
                                in_=band[ct].rearrange("s p q -> p s q"))
                            svals = [s for s in range(3)
                                     if 0 <= ct - 1 + s < T // P]
                            for si, s in enumerate(svals):
                                kt = ct - 1 + s
                                xa = wsm.tile([P, P], DT_R, name="wsm_t")
                                nc.sync.dma_start(
                                    out=xa,
                                    in_=xin[b, bass.ts(kt, P),
                                            bass.ts(fh, P)])
                                nc.tensor.matmul(
                                    pg[:, bass.ts(q, P)],
                                    xa,
                                    bt[:, s, :],
                                    start=(si == 0),
                                    stop=(si == len(svals) - 1))
                        nc.vector.tensor_copy(sf[:, fh, bass.ts(g4, 512)], pg)

                def pview(pt):
                    i, fh = pt // 2, pt % 2
                    return sf[:, fh, :].rearrange(
                        "p (s four) -> p four s", four=PH)[:, i, :]

                pn = pnp.tile([P, 8, 512], DT_R, name="pn")
                layer_norm_fm(pview, lambda d: pn[:, d, :], 8, PATCH, 512,
                              lambda d: lnp1g_t[:, d:d + 1],
                              lambda d: lnp1b_t[:, d:d + 1])

                for dt in range(DTILES):
                    wt = wsm.tile([P, KTILES, P], DT_R, name="wsm_t")
                    nc.sync.dma_start(
                        out=wt,
                        in_=par["wpe"].rearrange("(ko p) m -> p ko m", p=P)[
                            :, :, bass.ts(dt, P)])
                    pq = psm.tile([P, 512], DT_F, name="pmain")
                    for kt in range(KTILES):
                        nc.tensor.matmul(pq, wt[:, kt, :], pn[:, kt, :],
                                         start=(kt == 0), stop=(kt == 7))
                    nc.vector.tensor_scalar(
                        x0[:, dt, bass.ts(b, 512)], pq,
                        bpe_t[:, dt:dt + 1], None, OP.add)

            layer_norm_fm(lambda d: x0[:, d, :], lambda d: x[:, d, :],
                          DTILES, DIM, TOK,
                          lambda d: lnp2g_t[:, d:d + 1],
                          lambda d: lnp2b_t[:, d:d + 1])

        # =================== transformer layers ===================
        for l in range(DEPTH):
            h = big.tile([P, DTILES, TOK], DT_R, name="bigbuf")
            layer_norm_fm(lambda d: x[:, d, :], lambda d: h[:, d, :],
                          DTILES, DIM, TOK,
                          lambda d: lnag_t[:, l, d:d + 1],
                          lambda d: lnab_t[:, l, d:d + 1])

            et_sb = one1.tile([P, SEQT, SEQ], DT_F, name="et_sb")
            nc.sync.dma_start(out=et_sb,
                              in_=par["etab"][l].rearrange("jt p i -> p jt i"))

            of = big.tile([P, DTILES, TOK], DT_R, name="bigbuf")
            wq3 = par["wqkv"][l].rearrange("(ko p) m -> p ko m", p=P)
            for b in range(BPC):
                tsl = bass.ts(b, 512)
                with tc.tile_pool(name=f"qkv{l}_{b}", bufs=1) as qkvp:
                    qf = qkvp.tile([P, DTILES, 512], DT_R, name="qf")
                    kf = qkvp.tile([P, DTILES, 512], DT_R, name="kf")
                    vt = qkvp.tile([P, SEQT, INNER], DT_H, name="vt")
                    # q, k feature-major (k scaled by 1/sqrt(dhead))
                    for c in range(2 * DTILES):
                        wt = wsm.tile([P, KTILES, P], DT_R, name="wsm_t")
                        nc.sync.dma_start(out=wt, in_=wq3[:, :, bass.ts(c, P)])
                        pq = psm.tile([P, 512], DT_F, name="pmain")
                        for kt in range(KTILES):
                            nc.tensor.matmul(pq, wt[:, kt, :], h[:, kt, tsl],
                                             start=(kt == 0), stop=(kt == 7))
                        if c < DTILES:
                            nc.vector.tensor_copy(qf[:, c, :], pq)
                        else:
                            nc.vector.tensor_scalar(
                                kf[:, c - DTILES, :], pq,
                                float(DHEAD) ** -0.5, None, OP.mult)
                    # v token-major with ones column per head
                    for nh in range(2):
                        pvs = [psm.tile([P, 512], DT_F, name="pmain")
                               for _ in range(SEQT)]
                        for kt in range(KTILES):
                            wv = wsm.tile([P, 512], DT_R, name="wsm_t")
                            nc.sync.dma_start(
                                out=wv,
                                in_=wq3[:, kt, 2 * INNER + nh * 512:
                                        2 * INNER + (nh + 1) * 512])
                            for tt in range(SEQT):
                                nc.tensor.matmul(
                                    pvs[tt],
                                    h[:, kt, b * 512 + tt * P:
                                      b * 512 + (tt + 1) * P],
                                    wv,
                                    start=(kt == 0), stop=(kt == 7))
                        for tt in range(SEQT):
                            nc.vector.tensor_copy(
                                vt[:, tt, nh * 512:(nh + 1) * 512], pvs[tt])

                    for hd in range(HEADS):
                        po = (hd % 2) * 64
                        dt = hd // 2
                        sc = [psm.tile([P, 512], DT_F, name="pmain")
                              for _ in range(SEQT)]
                        for jt in range(SEQT):
                            nc.tensor.matmul(
                                sc[jt],
                                kf[po:po + 64, dt, bass.ts(jt, P)],
                                qf[po:po + 64, dt, :],
                                start=True, stop=True)
                        etr = sm2.tile([P, SEQT, 512], DT_H, name="etr")
                        for jt in range(SEQT):
                            ex = sm2.tile([P, 512], DT_F, name="ex")
                            nc.scalar.activation(ex, sc[jt], FX.Exp)
                            nc.vector.tensor_mul(
                                etr[:, jt, :], ex, et_sb[:, jt, :])
                        ot = pso.tile([P, 512], DT_F, name="ot")
                        den = pst.tile([1, 512], DT_F, name="ps0")
                        for jt in range(SEQT):
                            nc.tensor.matmul(
                                ot[0:64, :],
                                vt[:, jt, hd * 64:(hd + 1) * 64],
                                etr[:, jt, :],
                                start=(jt == 0), stop=(jt == SEQT - 1))
                            nc.tensor.matmul(
                                den, ones_h, etr[:, jt, :],
                                start=(jt == 0), stop=(jt == SEQT - 1))
                        adr1 = stats.tile([1, 512], DT_F, name="adr1")
                        nc.vector.reciprocal(adr1, den)
                        adrb = stats.tile([64, 512], DT_F, name="adrb")
                        nc.gpsimd.partition_broadcast(adrb, adr1)
                        if po == 0:
                            nc.vector.tensor_mul(
                                of[0:64, dt, tsl], ot[0:64, :], adrb)
                        else:
                            # DVE can't shift partitions: normalize at base 0,
                            # DMA-copy up to partitions 64:128.
                            otmp = sm2.tile([64, 512], DT_R, name="otmp")
                            nc.vector.tensor_mul(otmp, ot[0:64, :], adrb)
                            nc.sync.dma_start(
                                out=of[64:128, dt, tsl], in_=otmp)

            wo3 = par["wo"][l].rearrange("(ko p) m -> p ko m", p=P)
            for dt in range(DTILES):
                wt = wsm.tile([P, KTILES, P], DT_R, name="wsm_t")
                nc.sync.dma_start(out=wt, in_=wo3[:, :, bass.ts(dt, P)])
                for b in range(BPC):
                    pq = psm.tile([P, 512], DT_F, name="pmain")
                    for kt in range(KTILES):
                        nc.tensor.matmul(
                            pq, wt[:, kt, :], of[:, kt, bass.ts(b, 512)],
                            start=(kt == 0), stop=(kt == 7))
                    nc.scalar.activation(pq, pq, FX.Identity,
                                         bias=bov_t[:, l, dt:dt + 1])
                    nc.vector.tensor_add(
                        x[:, dt, bass.ts(b, 512)], pq,
                        x[:, dt, bass.ts(b, 512)])

            h2 = big.tile([P, DTILES, TOK], DT_R, name="bigbuf")
            layer_norm_fm(lambda d: x[:, d, :], lambda d: h2[:, d, :],
                          DTILES, DIM, TOK,
                          lambda d: lnfg_t[:, l, d:d + 1],
                          lambda d: lnfb_t[:, l, d:d + 1])
            w13 = par["w1"][l].rearrange("(ko p) m -> p ko m", p=P)
            w23 = par["w2"][l].rearrange("(ko p) m -> p ko m", p=P)
            for th in range(2):
                tsl = bass.ts(th, 512)
                with tc.tile_pool(name=f"h1p{l}_{th}", bufs=1) as h1p:
                    h1r = h1p.tile([P, MTILES, 512], DT_H, name="h1r")
                    for mt in range(MTILES):
                        wt = wsm.tile([P, KTILES, P], DT_R, name="wsm_t")
                        nc.sync.dma_start(out=wt,
                                          in_=w13[:, :, bass.ts(mt, P)])
                        pq = psm.tile([P, 512], DT_F, name="pmain")
                        for kt in range(KTILES):
                            nc.tensor.matmul(pq, wt[:, kt, :], h2[:, kt, tsl],
                                             start=(kt == 0), stop=(kt == 7))
                        nc.scalar.activation(
                            h1r[:, mt, :], pq, FX.Gelu,
                            bias=b1v_t[:, l, mt:mt + 1], scale=1.0)
                    for dt in range(DTILES):
                        pq = psm.tile([P, 512], DT_F, name="pmain")
                        for kh in range(2):
                            wt = wsm.tile([P, 16, P], DT_H, name="wsm_t")
                            nc.sync.dma_start(
                                out=wt,
                                in_=w23[:, kh * 16:(kh + 1) * 16,
                                        bass.ts(dt, P)])
                            for k2 in range(16):
                                kt = kh * 16 + k2
                                nc.tensor.matmul(
                                    pq, wt[:, k2, :], h1r[:, kt, :],
                                    start=(kt == 0), stop=(kt == 31))
                        nc.scalar.activation(pq, pq, FX.Identity,
                                             bias=b2v_t[:, l, dt:dt + 1])
                        nc.vector.tensor_add(
                            x[:, dt, tsl], pq, x[:, dt, tsl])

        # =================== head ===================
        ho = big.tile([P, DTILES, TOK], DT_R, name="bigbuf")
        layer_norm_fm(lambda d: x[:, d, :], lambda d: ho[:, d, :],
                      DTILES, DIM, TOK,
                      lambda d: lnog_t[:, d:d + 1],
                      lambda d: lnob_t[:, d:d + 1])
        wp3 = par["wproj"].rearrange("(ko p) m -> p ko m", p=P)
        wt = wsm.tile([P, KTILES, NCLS], DT_R, name="wsm_t")
        nc.sync.dma_start(out=wt, in_=wp3)
        out_sb = one1.tile([NCLS, TOK], DT_F, name="out_sb")
        for th in range(2):
            pq = pso.tile([P, 512], DT_F, name="ot")
            for kt in range(KTILES):
                nc.tensor.matmul(pq[0:NCLS, :], wt[:, kt, :],
                                 ho[:, kt, bass.ts(th, 512)],
                                 start=(kt == 0), stop=(kt == 7))
            nc.scalar.activation(out_sb[:, bass.ts(th, 512)], pq[0:NCLS, :],
                                 FX.Identity, bias=bproj_t)
        nc.sync.dma_start(out=par["out"][:, :], in_=out_sb)


# ============================================================
# host side
# ============================================================

_NC_CACHE = None


def _host_band():
    tt = np.arange(KSIZE, dtype=np.float64)
    kern = np.exp(-0.5 * ((tt - (KSIZE - 1) / 2.0) / SIGMA) ** 2)
    kern = (kern / kern.sum()).astype(np.float32)
    pad_l = (KSIZE - 1) // 2  # 9
    nt = T // P
    bandc = np.zeros((nt, 3, P, P), dtype=np.float32)
    for ct in range(nt):
        for s in range(3):
            kt = ct - 1 + s
            if not (0 <= kt < nt):
                continue
            rows = np.arange(kt * P, (kt + 1) * P)
            cols = np.arange(ct * P, (ct + 1) * P)
            d = rows[:, None] - cols[None, :] + pad_l
            m = (d >= 0) & (d < KSIZE)
            blk = np.zeros((P, P), np.float32)
            blk[m] = kern[d[m]]
            bandc[ct, s] = blk
    return bandc


def _host_etab(rel_tab):
    i = np.arange(SEQ)
    j = i[:, None]
    rel = np.clip(i[None, :] - j, -(MAXREL - 1), MAXREL - 1) + MAXREL - 1
    et = np.zeros((DEPTH, SEQ, SEQ), dtype=np.float32)
    for l in range(DEPTH):
        e = np.exp(rel_tab[l][rel])
        e[j > i[None, :]] = 0.0
        et[l] = e
    return et.reshape(DEPTH, SEQT, P, SEQ)


def kernel(**inputs):
    global _NC_CACHE
    if _NC_CACHE is None:
        _NC_CACHE = build_nc()
    nc = _NC_CACHE

    f32 = lambda a: np.ascontiguousarray(np.asarray(a, dtype=np.float32))
    import ml_dtypes
    shared = {
        "band": _host_band(),
        "etab": _host_etab(f32(inputs["rel_tab"])),
        "wpe": f32(inputs["W_pe"]),
        "wqkv": f32(inputs["Wqkv"]),
        "wo": f32(inputs["Wo"]),
        "w1": f32(inputs["W1"]),
        "w2": f32(inputs["W2"]).astype(ml_dtypes.bfloat16),
        "wproj": f32(inputs["Wproj"]),
        "lnp1g": f32(inputs["ln_p1_g"]), "lnp1b": f32(inputs["ln_p1_b"]),
        "bpe": f32(inputs["b_pe"]),
        "lnp2g": f32(inputs["ln_p2_g"]), "lnp2b": f32(inputs["ln_p2_b"]),
        "lnag": f32(inputs["ln_a_g"]), "lnab": f32(inputs["ln_a_b"]),
        "lnfg": f32(inputs["ln_f_g"]), "lnfb": f32(inputs["ln_f_b"]),
        "bov": f32(inputs["bo"]), "b1v": f32(inputs["b1"]),
        "b2v": f32(inputs["b2"]),
        "lnog": f32(inputs["ln_o_g"]), "lnob": f32(inputs["ln_o_b"]),
        "bprojv": f32(inputs["bproj"]),
    }
    xfull = f32(inputs["neuralInput"])
    in_maps = []
    for c in range(NCORES):
        m = dict(shared)
        m["xin"] = np.ascontiguousarray(xfull[c * BPC:(c + 1) * BPC])
        in_maps.append(m)

    import os
    trace = bool(os.environ.get("BIT_TRACE"))
    res = run_bass_kernel_spmd(nc, in_maps, list(range(NCORES)), trace=trace)
    if trace:
        globals()["LAST_RESULT"] = res
    outs = []
    for c in range(NCORES):
        o = res.results[c]["out"]              # [NCLS, TOK]
        o = o.reshape(NCLS, BPC, SEQ).transpose(1, 2, 0)
        outs.append(o)
    return np.concatenate(outs, axis=0).astype(np.float32)


# revision 21
# speedup vs baseline: 1.0023x; 1.0023x over previous
"""Trainium2 Bass kernel for nn_BiT_Phoneme (dense transformer).

Data-parallel: 16 batch elems / 8 cores = 2 per core; each core runs the
full network on its 2 sequences (1024 "tokens"). Activations are kept
feature-major ([dim on partitions, tokens on free]) so matmuls chain
without transposes. LayerNorm stats use ones-vector matmuls (partition
reduction). Softmax runs transposed (keys on partitions) as
exp(scores)*E with E = exp(rel_bias)*causal_mask precomputed on host;
denominators come from a ones column appended to V; normalization is a
reciprocal broadcast. Matmuls use float32r (full PE rate, ~1.5e-4);
attention probs / V / h1 / W2 use bf16.
"""

import numpy as np

import concourse.bass as bass
import concourse.mybir as mybir
import concourse.tile as tile
from concourse import bacc
from concourse.bass_utils import run_bass_kernel_spmd

B, T, F = 16, 2048, 256
PH = 4
PATCH = 1024
DIM = 1024
DEPTH = 6
HEADS, DHEAD = 16, 64
INNER = 1024
MLP = 4096
NCLS = 41
MAXREL = 200
KSIZE, SIGMA = 20, 2.0
EPS = 1e-5
SEQ = T // PH              # 512
NCORES = 8
BPC = B // NCORES          # 2
TOK = BPC * SEQ            # 1024
P = 128

DT_R = mybir.dt.float32r
DT_F = mybir.dt.float32
DT_H = mybir.dt.bfloat16
FX = mybir.ActivationFunctionType
OP = mybir.AluOpType

DTILES = DIM // P          # 8
KTILES = DIM // P          # 8
MTILES = MLP // P          # 32
SEQT = SEQ // P            # 4


def build_nc():
    nc = bacc.Bacc(None, target_bir_lowering=False)

    par = {}
    def dp(name, shape, dtype, is_out=False):
        par[name] = nc.declare_dram_parameter(name, list(shape), dtype, isOutput=is_out)
        return par[name]

    dp("xin", (BPC, T, F), DT_R)
    dp("band", (T // P, 3, P, P), DT_R)
    dp("etab", (DEPTH, SEQT, P, SEQ), DT_F)
    dp("wpe", (PATCH, DIM), DT_R)
    dp("wqkv", (DEPTH, DIM, 3 * INNER), DT_R)
    dp("wo", (DEPTH, INNER, DIM), DT_R)
    dp("w1", (DEPTH, DIM, MLP), DT_R)
    dp("w2", (DEPTH, MLP, DIM), DT_H)
    dp("wproj", (DIM, NCLS), DT_R)
    for nm, shp in [("lnp1g", (PATCH,)), ("lnp1b", (PATCH,)), ("bpe", (DIM,)),
                    ("lnp2g", (DIM,)), ("lnp2b", (DIM,)),
                    ("lnag", (DEPTH, DIM)), ("lnab", (DEPTH, DIM)),
                    ("lnfg", (DEPTH, DIM)), ("lnfb", (DEPTH, DIM)),
                    ("bov", (DEPTH, DIM)), ("b1v", (DEPTH, MLP)),
                    ("b2v", (DEPTH, DIM)), ("lnog", (DIM,)), ("lnob", (DIM,)),
                    ("bprojv", (NCLS,))]:
        dp(nm, shp, DT_F)
    dp("out", (NCLS, TOK), DT_F, is_out=True)

    with tile.TileContext(nc) as tc:
        _emit(nc, tc, par)
    nc.compile()
    return nc


def _emit(nc, tc, par):
    import contextlib
    ctx = contextlib.ExitStack()
    with ctx:
        const = ctx.enter_context(tc.tile_pool(name="const", bufs=1))
        xpool = ctx.enter_context(tc.tile_pool(name="xpool", bufs=1))
        big = ctx.enter_context(tc.tile_pool(name="big", bufs=2))
        wsm = ctx.enter_context(tc.tile_pool(name="wsm", bufs=3))
        stats = ctx.enter_context(tc.tile_pool(name="stats", bufs=1))
        sm2 = ctx.enter_context(tc.tile_pool(name="sm2", bufs=2))
        one1 = ctx.enter_context(tc.tile_pool(name="one1", bufs=1))
        psm = ctx.enter_context(tc.tile_pool(name="psm", bufs=4, space="PSUM"))
        pso = ctx.enter_context(tc.tile_pool(name="pso", bufs=2, space="PSUM"))
        pst = ctx.enter_context(tc.tile_pool(name="pst", bufs=1, space="PSUM"))

        ones_r = const.tile([P, 1], DT_R, name="ones_r")
        nc.vector.memset(ones_r.bitcast(mybir.dt.uint32), 0x3F800000)
        ones_h = const.tile([P, 1], DT_H, name="ones_h")
        nc.vector.memset(ones_h.bitcast(mybir.dt.uint16), 0x3F80)
        epst = const.tile([1, 1], DT_F, name="epst")
        nc.vector.memset(epst, EPS)

        def load_vec(nm, width):
            d = par[nm]
            if len(d.shape) == 1:
                tl = const.tile([P, width // P], DT_F, name=nm + "_t")
                nc.sync.dma_start(out=tl, in_=d.rearrange("(o p) -> p o", p=P))
            else:
                L = d.shape[0]
                tl = const.tile([P, L, width // P], DT_F, name=nm + "_t")
                nc.sync.dma_start(out=tl, in_=d.rearrange("l (o p) -> p l o", p=P))
            return tl

        lnp1g_t = load_vec("lnp1g", PATCH)
        lnp1b_t = load_vec("lnp1b", PATCH)
        bpe_t = load_vec("bpe", DIM)
        lnp2g_t = load_vec("lnp2g", DIM)
        lnp2b_t = load_vec("lnp2b", DIM)
        lnag_t = load_vec("lnag", DIM)
        lnab_t = load_vec("lnab", DIM)
        lnfg_t = load_vec("lnfg", DIM)
        lnfb_t = load_vec("lnfb", DIM)
        bov_t = load_vec("bov", DIM)
        b1v_t = load_vec("b1v", MLP)
        b2v_t = load_vec("b2v", DIM)
        lnog_t = load_vec("lnog", DIM)
        lnob_t = load_vec("lnob", DIM)
        bproj_t = const.tile([NCLS, 1], DT_F, name="bproj_t")
        nc.sync.dma_start(out=bproj_t,
                          in_=par["bprojv"].rearrange("(p o) -> p o", o=1))

        x = xpool.tile([P, DTILES, TOK], DT_R, name="x")

        # ---- layernorm over partitions+tiles (feature-major) ----
        # views(d) -> [P, width] fp32r source slices (ntiles of them);
        # writes dst_fn(d) slices, width tokens, nh 512-halves.
        def layer_norm_fm(views, dst_fn, ntiles, D, width, g_fn, b_fn):
            nh = width // 512
            st = stats.tile([1, 2, TOK], DT_F, name="st")
            mu, rstd = st[:, 0, :width], st[:, 1, :width]
            for th in range(nh):
                sl = bass.ts(th, 512)
                ps0 = pst.tile([1, 512], DT_F, name="ps0")
                ps1 = pst.tile([1, 512], DT_F, name="ps1")
                for d in range(ntiles):
                    v = views(d)[:, sl]
                    sq = sm2.tile([P, 512], DT_R, name="sq")
                    nc.vector.tensor_mul(sq, v, v)
                    nc.tensor.matmul(ps0, ones_r, v,
                                     start=(d == 0), stop=(d == ntiles - 1))
                    nc.tensor.matmul(ps1, ones_r, sq,
                                     start=(d == 0), stop=(d == ntiles - 1))
                nc.scalar.mul(mu[:, sl], ps0, 1.0 / D)
                nc.scalar.mul(ps1, ps1, 1.0 / D)
                # rstd slot <- mu^2, then var = E[x^2] - mu^2
                nc.scalar.activation(rstd[:, sl], mu[:, sl], FX.Square)
                nc.vector.tensor_sub(rstd[:, sl], ps1, rstd[:, sl])
            nc.scalar.activation(rstd, rstd, FX.Sqrt, bias=epst, scale=1.0)
            nc.vector.reciprocal(rstd, rstd)
            mrb = stats.tile([P, 2, TOK], DT_F, name="mrb")
            nc.gpsimd.partition_broadcast(mrb[:, 0, :width], mu)
            nc.gpsimd.partition_broadcast(mrb[:, 1, :width], rstd)
            for d in range(ntiles):
                for th in range(nh):
                    sl = bass.ts(th, 512)
                    tmp = sm2.tile([P, 512], DT_F, name="lntmp")
                    nc.vector.tensor_sub(tmp, views(d)[:, sl], mrb[:, 0, sl])
                    nc.vector.tensor_mul(tmp, tmp, mrb[:, 1, sl])
                    nc.vector.tensor_scalar(
                        dst_fn(d)[:, sl], tmp, g_fn(d), b_fn(d),
                        OP.mult, OP.add)

        # =================== embedding ===================
        xin, band = par["xin"], par["band"]
        with (
            tc.tile_pool(name="sfp", bufs=1) as sfp,
            tc.tile_pool(name="pnp", bufs=1) as pnp,
        ):
            x0 = big.tile([P, DTILES, TOK], DT_R, name="bigbuf")
            for b in range(BPC):
                sf = sfp.tile([P, 2, T], DT_R, name="sf")
                for fh in range(2):
                    for g4 in range(T // 512):
                        pg = psm.tile([P, 512], DT_F, name="pmain")
                        for q in range(4):
                            ct = g4 * 4 + q
                            bt = wsm.tile([P, 3, P], DT_R, name="wsm_t")
                            nc.sync.dma_start(
                                out=bt,
                                in_=band[ct].rearrange("s p q -> p s q"))
                            svals = [s for s in range(3)
                                     if 0 <= ct - 1 + s < T // P]
                            for si, s in enumerate(svals):
                                kt = ct - 1 + s
                                xa = wsm.tile([P, P], DT_R, name="wsm_t")
                                nc.sync.dma_start(
                                    out=xa,
                                    in_=xin[b, bass.ts(kt, P),
                                            bass.ts(fh, P)])
                                nc.tensor.matmul(
                                    pg[:, bass.ts(q, P)],
                                    xa,
                                    bt[:, s, :],
                                    start=(si == 0),
                                    stop=(si == len(svals) - 1))
                        nc.vector.tensor_copy(sf[:, fh, bass.ts(g4, 512)], pg)

                def pview(pt):
                    i, fh = pt // 2, pt % 2
                    return sf[:, fh, :].rearrange(
                        "p (s four) -> p four s", four=PH)[:, i, :]

                pn = pnp.tile([P, 8, 512], DT_R, name="pn")
                layer_norm_fm(pview, lambda d: pn[:, d, :], 8, PATCH, 512,
                              lambda d: lnp1g_t[:, d:d + 1],
                              lambda d: lnp1b_t[:, d:d + 1])

                for dt in range(DTILES):
                    wt = wsm.tile([P, KTILES, P], DT_R, name="wsm_t")
                    nc.sync.dma_start(
                        out=wt,
                        in_=par["wpe"].rearrange("(ko p) m -> p ko m", p=P)[
                            :, :, bass.ts(dt, P)])
                    pq = psm.tile([P, 512], DT_F, name="pmain")
                    for kt in range(KTILES):
                        nc.tensor.matmul(pq, wt[:, kt, :], pn[:, kt, :],
                                         start=(kt == 0), stop=(kt == 7))
                    nc.vector.tensor_scalar(
                        x0[:, dt, bass.ts(b, 512)], pq,
                        bpe_t[:, dt:dt + 1], None, OP.add)

            layer_norm_fm(lambda d: x0[:, d, :], lambda d: x[:, d, :],
                          DTILES, DIM, TOK,
                          lambda d: lnp2g_t[:, d:d + 1],
                          lambda d: lnp2b_t[:, d:d + 1])

        # =================== transformer layers ===================
        for l in range(DEPTH):
            h = big.tile([P, DTILES, TOK], DT_R, name="bigbuf")
            with nc.named_scope(f"L{l}_lna"):
                layer_norm_fm(lambda d: x[:, d, :], lambda d: h[:, d, :],
                              DTILES, DIM, TOK,
                              lambda d: lnag_t[:, l, d:d + 1],
                              lambda d: lnab_t[:, l, d:d + 1])

            et_sb = one1.tile([P, SEQT, SEQ], DT_F, name="et_sb")
            nc.sync.dma_start(out=et_sb,
                              in_=par["etab"][l].rearrange("jt p i -> p jt i"))

            of = big.tile([P, DTILES, TOK], DT_R, name="bigbuf")
            wq3 = par["wqkv"][l].rearrange("(ko p) m -> p ko m", p=P)
            for b in range(BPC):
                tsl = bass.ts(b, 512)
                with tc.tile_pool(name=f"qkv{l}_{b}", bufs=1) as qkvp:
                    qf = qkvp.tile([P, DTILES, 512], DT_R, name="qf")
                    kf = qkvp.tile([P, DTILES, 512], DT_R, name="kf")
                    vt = qkvp.tile([P, SEQT, INNER], DT_H, name="vt")
                    # q, k feature-major (k scaled by 1/sqrt(dhead))
                    for c in range(2 * DTILES):
                        wt = wsm.tile([P, KTILES, P], DT_R, name="wsm_t")
                        nc.sync.dma_start(out=wt, in_=wq3[:, :, bass.ts(c, P)])
                        pq = psm.tile([P, 512], DT_F, name="pmain")
                        for kt in range(KTILES):
                            nc.tensor.matmul(pq, wt[:, kt, :], h[:, kt, tsl],
                                             start=(kt == 0), stop=(kt == 7))
                        if c < DTILES:
                            nc.vector.tensor_copy(qf[:, c, :], pq)
                        else:
                            nc.vector.tensor_scalar(
                                kf[:, c - DTILES, :], pq,
                                float(DHEAD) ** -0.5, None, OP.mult)
                    # v token-major with ones column per head
                    for nh in range(2):
                        pvs = [psm.tile([P, 512], DT_F, name="pmain")
                               for _ in range(SEQT)]
                        for kt in range(KTILES):
                            wv = wsm.tile([P, 512], DT_R, name="wsm_t")
                            nc.sync.dma_start(
                                out=wv,
                                in_=wq3[:, kt, 2 * INNER + nh * 512:
                                        2 * INNER + (nh + 1) * 512])
                            for tt in range(SEQT):
                                nc.tensor.matmul(
                                    pvs[tt],
                                    h[:, kt, b * 512 + tt * P:
                                      b * 512 + (tt + 1) * P],
                                    wv,
                                    start=(kt == 0), stop=(kt == 7))
                        for tt in range(SEQT):
                            nc.vector.tensor_copy(
                                vt[:, tt, nh * 512:(nh + 1) * 512], pvs[tt])

                    for hd in range(HEADS):
                        po = (hd % 2) * 64
                        dt = hd // 2
                        sc = [psm.tile([P, 512], DT_F, name="pmain")
                              for _ in range(SEQT)]
                        for jt in range(SEQT):
                            nc.tensor.matmul(
                                sc[jt],
                                kf[po:po + 64, dt, bass.ts(jt, P)],
                                qf[po:po + 64, dt, :],
                                start=True, stop=True)
                        etr = sm2.tile([P, SEQT, 512], DT_H, name="etr")
                        for jt in range(SEQT):
                            ex = sm2.tile([P, 512], DT_F, name="ex")
                            nc.scalar.activation(ex, sc[jt], FX.Exp)
                            nc.vector.tensor_mul(
                                etr[:, jt, :], ex, et_sb[:, jt, :])
                        ot = pso.tile([P, 512], DT_F, name="ot")
                        den = pst.tile([1, 512], DT_F, name="ps0")
                        for jt in range(SEQT):
                            nc.tensor.matmul(
                                ot[0:64, :],
                                vt[:, jt, hd * 64:(hd + 1) * 64],
                                etr[:, jt, :],
                                start=(jt == 0), stop=(jt == SEQT - 1))
                            nc.tensor.matmul(
                                den, ones_h, etr[:, jt, :],
                                start=(jt == 0), stop=(jt == SEQT - 1))
                        adr1 = stats.tile([1, 512], DT_F, name="adr1")
                        nc.vector.reciprocal(adr1, den)
                        adrb = stats.tile([64, 512], DT_F, name="adrb")
                        nc.gpsimd.partition_broadcast(adrb, adr1)
                        if po == 0:
                            nc.vector.tensor_mul(
                                of[0:64, dt, tsl], ot[0:64, :], adrb)
                        else:
                            # DVE can't shift partitions: normalize at base 0,
                            # DMA-copy up to partitions 64:128.
                            otmp = sm2.tile([64, 512], DT_R, name="otmp")
                            nc.vector.tensor_mul(otmp, ot[0:64, :], adrb)
                            nc.sync.dma_start(
                                out=of[64:128, dt, tsl], in_=otmp)

            wo3 = par["wo"][l].rearrange("(ko p) m -> p ko m", p=P)
            for dt in range(DTILES):
                wt = wsm.tile([P, KTILES, P], DT_R, name="wsm_t")
                nc.sync.dma_start(out=wt, in_=wo3[:, :, bass.ts(dt, P)])
                for b in range(BPC):
                    pq = psm.tile([P, 512], DT_F, name="pmain")
                    for kt in range(KTILES):
                        nc.tensor.matmul(
                            pq, wt[:, kt, :], of[:, kt, bass.ts(b, 512)],
                            start=(kt == 0), stop=(kt == 7))
                    nc.scalar.activation(pq, pq, FX.Identity,
                                         bias=bov_t[:, l, dt:dt + 1])
                    nc.vector.tensor_add(
                        x[:, dt, bass.ts(b, 512)], pq,
                        x[:, dt, bass.ts(b, 512)])

            h2 = big.tile([P, DTILES, TOK], DT_R, name="bigbuf")
            layer_norm_fm(lambda d: x[:, d, :], lambda d: h2[:, d, :],
                          DTILES, DIM, TOK,
                          lambda d: lnfg_t[:, l, d:d + 1],
                          lambda d: lnfb_t[:, l, d:d + 1])
            w13 = par["w1"][l].rearrange("(ko p) m -> p ko m", p=P)
            w23 = par["w2"][l].rearrange("(ko p) m -> p ko m", p=P)
            for th in range(2):
                tsl = bass.ts(th, 512)
                with tc.tile_pool(name=f"h1p{l}_{th}", bufs=1) as h1p:
                    h1r = h1p.tile([P, MTILES, 512], DT_H, name="h1r")
                    for mt in range(MTILES):
                        wt = wsm.tile([P, KTILES, P], DT_R, name="wsm_t")
                        nc.sync.dma_start(out=wt,
                                          in_=w13[:, :, bass.ts(mt, P)])
                        pq = psm.tile([P, 512], DT_F, name="pmain")
                        for kt in range(KTILES):
                            nc.tensor.matmul(pq, wt[:, kt, :], h2[:, kt, tsl],
                                             start=(kt == 0), stop=(kt == 7))
                        nc.scalar.activation(
                            h1r[:, mt, :], pq, FX.Gelu,
                            bias=b1v_t[:, l, mt:mt + 1], scale=1.0)
                    for dt in range(DTILES):
                        pq = psm.tile([P, 512], DT_F, name="pmain")
                        for kh in range(2):
                            wt = wsm.tile([P, 16, P], DT_H, name="wsm_t")
                            nc.sync.dma_start(
                                out=wt,
                                in_=w23[:, kh * 16:(kh + 1) * 16,
                                        bass.ts(dt, P)])
                            for k2 in range(16):
                                kt = kh * 16 + k2
                                nc.tensor.matmul(
                                    pq, wt[:, k2, :], h1r[:, kt, :],
                                    start=(kt == 0), stop=(kt == 31))
                        nc.scalar.activation(pq, pq, FX.Identity,
                                             bias=b2v_t[:, l, dt:dt + 1])
                        nc.vector.tensor_add(
                            x[:, dt, tsl], pq, x[:, dt, tsl])

        # =================== head ===================
        ho = big.tile([P, DTILES, TOK], DT_R, name="bigbuf")
        layer_norm_fm(lambda d: x[:, d, :], lambda d: ho[:, d, :],
                      DTILES, DIM, TOK,
                      lambda d: lnog_t[:, d:d + 1],
                      lambda d: lnob_t[:, d:d + 1])
        wp3 = par["wproj"].rearrange("(ko p) m -> p ko m", p=P)
        wt = wsm.tile([P, KTILES, NCLS], DT_R, name="wsm_t")
        nc.sync.dma_start(out=wt, in_=wp3)
        out_sb = one1.tile([NCLS, TOK], DT_F, name="out_sb")
        for th in range(2):
            pq = pso.tile([P, 512], DT_F, name="ot")
            for kt in range(KTILES):
                nc.tensor.matmul(pq[0:NCLS, :], wt[:, kt, :],
                                 ho[:, kt, bass.ts(th, 512)],
                                 start=(kt == 0), stop=(kt == 7))
            nc.scalar.activation(out_sb[:, bass.ts(th, 512)], pq[0:NCLS, :],
                                 FX.Identity, bias=bproj_t)
        nc.sync.dma_start(out=par["out"][:, :], in_=out_sb)


# ============================================================
# host side
# ============================================================

_NC_CACHE = None


def _host_band():
    tt = np.arange(KSIZE, dtype=np.float64)
    kern = np.exp(-0.5 * ((tt - (KSIZE - 1) / 2.0) / SIGMA) ** 2)
    kern = (kern / kern.sum()).astype(np.float32)
    pad_l = (KSIZE - 1) // 2  # 9
    nt = T // P
    bandc = np.zeros((nt, 3, P, P), dtype=np.float32)
    for ct in range(nt):
        for s in range(3):
            kt = ct - 1 + s
            if not (0 <= kt < nt):
                continue
            rows = np.arange(kt * P, (kt + 1) * P)
            cols = np.arange(ct * P, (ct + 1) * P)
            d = rows[:, None] - cols[None, :] + pad_l
            m = (d >= 0) & (d < KSIZE)
            blk = np.zeros((P, P), np.float32)
            blk[m] = kern[d[m]]
            bandc[ct, s] = blk
    return bandc


def _host_etab(rel_tab):
    i = np.arange(SEQ)
    j = i[:, None]
    rel = np.clip(i[None, :] - j, -(MAXREL - 1), MAXREL - 1) + MAXREL - 1
    et = np.zeros((DEPTH, SEQ, SEQ), dtype=np.float32)
    for l in range(DEPTH):
        e = np.exp(rel_tab[l][rel])
        e[j > i[None, :]] = 0.0
        et[l] = e
    return et.reshape(DEPTH, SEQT, P, SEQ)


def kernel(**inputs):
    global _NC_CACHE
    if _NC_CACHE is None:
        _NC_CACHE = build_nc()
    nc = _NC_CACHE

    f32 = lambda a: np.ascontiguousarray(np.asarray(a, dtype=np.float32))
    import ml_dtypes
    shared = {
        "band": _host_band(),
        "etab": _host_etab(f32(inputs["rel_tab"])),
        "wpe": f32(inputs["W_pe"]),
        "wqkv": f32(inputs["Wqkv"]),
        "wo": f32(inputs["Wo"]),
        "w1": f32(inputs["W1"]),
        "w2": f32(inputs["W2"]).astype(ml_dtypes.bfloat16),
        "wproj": f32(inputs["Wproj"]),
        "lnp1g": f32(inputs["ln_p1_g"]), "lnp1b": f32(inputs["ln_p1_b"]),
        "bpe": f32(inputs["b_pe"]),
        "lnp2g": f32(inputs["ln_p2_g"]), "lnp2b": f32(inputs["ln_p2_b"]),
        "lnag": f32(inputs["ln_a_g"]), "lnab": f32(inputs["ln_a_b"]),
        "lnfg": f32(inputs["ln_f_g"]), "lnfb": f32(inputs["ln_f_b"]),
        "bov": f32(inputs["bo"]), "b1v": f32(inputs["b1"]),
        "b2v": f32(inputs["b2"]),
        "lnog": f32(inputs["ln_o_g"]), "lnob": f32(inputs["ln_o_b"]),
        "bprojv": f32(inputs["bproj"]),
    }
    xfull = f32(inputs["neuralInput"])
    in_maps = []
    for c in range(NCORES):
        m = dict(shared)
        m["xin"] = np.ascontiguousarray(xfull[c * BPC:(c + 1) * BPC])
        in_maps.append(m)

    import os
    trace = bool(os.environ.get("BIT_TRACE"))
    res = run_bass_kernel_spmd(nc, in_maps, list(range(NCORES)), trace=trace)
    if trace:
        globals()["LAST_RESULT"] = res
    outs = []
    for c in range(NCORES):
        o = res.results[c]["out"]              # [NCLS, TOK]
        o = o.reshape(NCLS, BPC, SEQ).transpose(1, 2, 0)
        outs.append(o)
    return np.concatenate(outs, axis=0).astype(np.float32)


# revision 26
# speedup vs baseline: 1.0599x; 1.0575x over previous
"""Trainium2 Bass kernel for nn_BiT_Phoneme (dense transformer).

Data-parallel: 16 batch elems / 8 cores = 2 per core; each core runs the
full network on its 2 sequences (1024 "tokens"). Activations are kept
feature-major ([dim on partitions, tokens on free]) so matmuls chain
without transposes. LayerNorm stats use ones-vector matmuls (partition
reduction). Softmax runs transposed (keys on partitions) as
exp(scores)*E with E = exp(rel_bias)*causal_mask precomputed on host;
denominators come from a ones column appended to V; normalization is a
reciprocal broadcast. Matmuls use float32r (full PE rate, ~1.5e-4);
attention probs / V / h1 / W2 use bf16.
"""

import numpy as np

import concourse.bass as bass
import concourse.mybir as mybir
import concourse.tile as tile
from concourse import bacc
from concourse.bass_utils import run_bass_kernel_spmd

B, T, F = 16, 2048, 256
PH = 4
PATCH = 1024
DIM = 1024
DEPTH = 6
HEADS, DHEAD = 16, 64
INNER = 1024
MLP = 4096
NCLS = 41
MAXREL = 200
KSIZE, SIGMA = 20, 2.0
EPS = 1e-5
SEQ = T // PH              # 512
NCORES = 8
BPC = B // NCORES          # 2
TOK = BPC * SEQ            # 1024
P = 128

DT_R = mybir.dt.float32r
DT_F = mybir.dt.float32
DT_H = mybir.dt.bfloat16
FX = mybir.ActivationFunctionType
OP = mybir.AluOpType

DTILES = DIM // P          # 8
KTILES = DIM // P          # 8
MTILES = MLP // P          # 32
SEQT = SEQ // P            # 4


def build_nc():
    nc = bacc.Bacc(None, target_bir_lowering=False)

    par = {}
    def dp(name, shape, dtype, is_out=False):
        par[name] = nc.declare_dram_parameter(name, list(shape), dtype, isOutput=is_out)
        return par[name]

    dp("xin", (BPC, T, F), DT_R)
    dp("band", (T // P, 3, P, P), DT_R)
    dp("etab", (DEPTH, SEQT, P, SEQ), DT_F)
    dp("wpe", (PATCH, DIM), DT_R)
    dp("wqkv", (DEPTH, DIM, 3 * INNER), DT_R)
    dp("wo", (DEPTH, INNER, DIM), DT_R)
    dp("w1", (DEPTH, DIM, MLP), DT_R)
    dp("w2", (DEPTH, MLP, DIM), DT_H)
    dp("wproj", (DIM, NCLS), DT_R)
    for nm, shp in [("lnp1g", (PATCH,)), ("lnp1b", (PATCH,)), ("bpe", (DIM,)),
                    ("lnp2g", (DIM,)), ("lnp2b", (DIM,)),
                    ("lnag", (DEPTH, DIM)), ("lnab", (DEPTH, DIM)),
                    ("lnfg", (DEPTH, DIM)), ("lnfb", (DEPTH, DIM)),
                    ("bov", (DEPTH, DIM)), ("b1v", (DEPTH, MLP)),
                    ("b2v", (DEPTH, DIM)), ("lnog", (DIM,)), ("lnob", (DIM,)),
                    ("bprojv", (NCLS,))]:
        dp(nm, shp, DT_F)
    dp("out", (NCLS, TOK), DT_F, is_out=True)

    with tile.TileContext(nc) as tc:
        _emit(nc, tc, par)
    nc.compile()
    return nc


def _emit(nc, tc, par):
    import contextlib
    ctx = contextlib.ExitStack()
    with ctx:
        const = ctx.enter_context(tc.tile_pool(name="const", bufs=1))
        xpool = ctx.enter_context(tc.tile_pool(name="xpool", bufs=1))
        big = ctx.enter_context(tc.tile_pool(name="big", bufs=2))
        wsm = ctx.enter_context(tc.tile_pool(name="wsm", bufs=3))
        stats = ctx.enter_context(tc.tile_pool(name="stats", bufs=1))
        sm2 = ctx.enter_context(tc.tile_pool(name="sm2", bufs=2))
        one1 = ctx.enter_context(tc.tile_pool(name="one1", bufs=1))
        psm = ctx.enter_context(tc.tile_pool(name="psm", bufs=4, space="PSUM"))
        pso = ctx.enter_context(tc.tile_pool(name="pso", bufs=2, space="PSUM"))
        pst = ctx.enter_context(tc.tile_pool(name="pst", bufs=1, space="PSUM"))

        ones_r = const.tile([P, 1], DT_R, name="ones_r")
        nc.vector.memset(ones_r.bitcast(mybir.dt.uint32), 0x3F800000)
        ones_h = const.tile([P, 1], DT_H, name="ones_h")
        nc.vector.memset(ones_h.bitcast(mybir.dt.uint16), 0x3F80)
        ones_row = const.tile([1, P], DT_R, name="ones_row")
        nc.vector.memset(ones_row.bitcast(mybir.dt.uint32), 0x3F800000)
        epst = const.tile([1, 1], DT_F, name="epst")
        nc.vector.memset(epst, EPS)

        def load_vec(nm, width):
            d = par[nm]
            if len(d.shape) == 1:
                tl = const.tile([P, width // P], DT_F, name=nm + "_t")
                nc.sync.dma_start(out=tl, in_=d.rearrange("(o p) -> p o", p=P))
            else:
                L = d.shape[0]
                tl = const.tile([P, L, width // P], DT_F, name=nm + "_t")
                nc.sync.dma_start(out=tl, in_=d.rearrange("l (o p) -> p l o", p=P))
            return tl

        lnp1g_t = load_vec("lnp1g", PATCH)
        lnp1b_t = load_vec("lnp1b", PATCH)
        bpe_t = load_vec("bpe", DIM)
        lnp2g_t = load_vec("lnp2g", DIM)
        lnp2b_t = load_vec("lnp2b", DIM)
        lnag_t = load_vec("lnag", DIM)
        lnab_t = load_vec("lnab", DIM)
        lnfg_t = load_vec("lnfg", DIM)
        lnfb_t = load_vec("lnfb", DIM)
        bov_t = load_vec("bov", DIM)
        b1v_t = load_vec("b1v", MLP)
        b2v_t = load_vec("b2v", DIM)
        lnog_t = load_vec("lnog", DIM)
        lnob_t = load_vec("lnob", DIM)
        bproj_t = const.tile([NCLS, 1], DT_F, name="bproj_t")
        nc.sync.dma_start(out=bproj_t,
                          in_=par["bprojv"].rearrange("(p o) -> p o", o=1))

        x = xpool.tile([P, DTILES, TOK], DT_R, name="x")

        # ---- layernorm over partitions+tiles (feature-major) ----
        # views(d) -> [P, width] fp32r source slices (ntiles of them);
        # writes dst_fn(d) slices, width tokens, nh 512-halves.
        def layer_norm_fm(views, dst_fn, ntiles, D, width, g_fn, b_fn):
            nh = width // 512
            st = stats.tile([1, 2, TOK], DT_R, name="st")
            mu, rstd = st[:, 0, :width], st[:, 1, :width]
            vt32 = stats.tile([1, TOK], DT_F, name="vt32")
            vtmp = vt32[:, :width]
            for th in range(nh):
                sl = bass.ts(th, 512)
                ps0 = pst.tile([1, 512], DT_F, name="ps0")
                ps1 = pst.tile([1, 512], DT_F, name="ps1")
                for d in range(ntiles):
                    v = views(d)[:, sl]
                    sq = sm2.tile([P, 512], DT_R, name="sq")
                    nc.vector.tensor_mul(sq, v, v)
                    nc.tensor.matmul(ps0, ones_r, v,
                                     start=(d == 0), stop=(d == ntiles - 1))
                    nc.tensor.matmul(ps1, ones_r, sq,
                                     start=(d == 0), stop=(d == ntiles - 1))
                nc.vector.tensor_scalar(mu[:, sl], ps0, 1.0 / D, None, OP.mult)
                # rstd slot <- mu^2 (scratch), vt32 <- E[x^2] - mu^2
                nc.vector.tensor_scalar(vtmp[:, sl], ps1, 1.0 / D, None,
                                        OP.mult)
                nc.vector.tensor_mul(rstd[:, sl], mu[:, sl], mu[:, sl])
                nc.vector.tensor_sub(vtmp[:, sl], vtmp[:, sl], rstd[:, sl])
            nc.scalar.activation(vtmp, vtmp, FX.Sqrt, bias=epst, scale=1.0)
            with nc.allow_low_precision(reason="fp32r rstd feeds matmul bcast"):
                nc.vector.reciprocal(rstd, vtmp)
            for th in range(nh):
                sl = bass.ts(th, 512)
                # broadcast mu/rstd across partitions via K=1 matmul
                mps = psm.tile([P, 512], DT_F, name="pmain")
                rps = psm.tile([P, 512], DT_F, name="pmain")
                nc.tensor.matmul(mps, ones_row, mu[:, sl],
                                 start=True, stop=True)
                nc.tensor.matmul(rps, ones_row, rstd[:, sl],
                                 start=True, stop=True)
                for d in range(ntiles):
                    tmp = sm2.tile([P, 512], DT_F, name="lntmp")
                    nc.vector.tensor_sub(tmp, views(d)[:, sl], mps)
                    nc.vector.tensor_mul(tmp, tmp, rps)
                    nc.vector.tensor_scalar(
                        dst_fn(d)[:, sl], tmp, g_fn(d), b_fn(d),
                        OP.mult, OP.add)

        # =================== embedding ===================
        xin, band = par["xin"], par["band"]
        with (
            tc.tile_pool(name="sfp", bufs=1) as sfp,
            tc.tile_pool(name="pnp", bufs=1) as pnp,
        ):
            x0 = big.tile([P, DTILES, TOK], DT_R, name="bigbuf")
            for b in range(BPC):
                sf = sfp.tile([P, 2, T], DT_R, name="sf")
                for fh in range(2):
                    for g4 in range(T // 512):
                        pg = psm.tile([P, 512], DT_F, name="pmain")
                        for q in range(4):
                            ct = g4 * 4 + q
                            bt = wsm.tile([P, 3, P], DT_R, name="wsm_t")
                            nc.sync.dma_start(
                                out=bt,
                                in_=band[ct].rearrange("s p q -> p s q"))
                            svals = [s for s in range(3)
                                     if 0 <= ct - 1 + s < T // P]
                            for si, s in enumerate(svals):
                                kt = ct - 1 + s
                                xa = wsm.tile([P, P], DT_R, name="wsm_t")
                                nc.sync.dma_start(
                                    out=xa,
                                    in_=xin[b, bass.ts(kt, P),
                                            bass.ts(fh, P)])
                                nc.tensor.matmul(
                                    pg[:, bass.ts(q, P)],
                                    xa,
                                    bt[:, s, :],
                                    start=(si == 0),
                                    stop=(si == len(svals) - 1))
                        nc.vector.tensor_copy(sf[:, fh, bass.ts(g4, 512)], pg)

                def pview(pt):
                    i, fh = pt // 2, pt % 2
                    return sf[:, fh, :].rearrange(
                        "p (s four) -> p four s", four=PH)[:, i, :]

                pn = pnp.tile([P, 8, 512], DT_R, name="pn")
                layer_norm_fm(pview, lambda d: pn[:, d, :], 8, PATCH, 512,
                              lambda d: lnp1g_t[:, d:d + 1],
                              lambda d: lnp1b_t[:, d:d + 1])

                for dt in range(DTILES):
                    wt = wsm.tile([P, KTILES, P], DT_R, name="wsm_t")
                    nc.sync.dma_start(
                        out=wt,
                        in_=par["wpe"].rearrange("(ko p) m -> p ko m", p=P)[
                            :, :, bass.ts(dt, P)])
                    pq = psm.tile([P, 512], DT_F, name="pmain")
                    for kt in range(KTILES):
                        nc.tensor.matmul(pq, wt[:, kt, :], pn[:, kt, :],
                                         start=(kt == 0), stop=(kt == 7))
                    nc.vector.tensor_scalar(
                        x0[:, dt, bass.ts(b, 512)], pq,
                        bpe_t[:, dt:dt + 1], None, OP.add)

            layer_norm_fm(lambda d: x0[:, d, :], lambda d: x[:, d, :],
                          DTILES, DIM, TOK,
                          lambda d: lnp2g_t[:, d:d + 1],
                          lambda d: lnp2b_t[:, d:d + 1])

        # =================== transformer layers ===================
        for l in range(DEPTH):
            h = big.tile([P, DTILES, TOK], DT_R, name="bigbuf")
            with nc.named_scope(f"L{l}_lna"):
                layer_norm_fm(lambda d: x[:, d, :], lambda d: h[:, d, :],
                              DTILES, DIM, TOK,
                              lambda d: lnag_t[:, l, d:d + 1],
                              lambda d: lnab_t[:, l, d:d + 1])

            et_sb = one1.tile([P, SEQT, SEQ], DT_F, name="et_sb")
            nc.sync.dma_start(out=et_sb,
                              in_=par["etab"][l].rearrange("jt p i -> p jt i"))

            of = big.tile([P, DTILES, TOK], DT_R, name="bigbuf")
            wq3 = par["wqkv"][l].rearrange("(ko p) m -> p ko m", p=P)
            for b in range(BPC):
                tsl = bass.ts(b, 512)
                with tc.tile_pool(name=f"qkv{l}_{b}", bufs=1) as qkvp:
                    qf = qkvp.tile([P, DTILES, 512], DT_R, name="qf")
                    kf = qkvp.tile([P, DTILES, 512], DT_R, name="kf")
                    vt = qkvp.tile([P, SEQT, INNER], DT_H, name="vt")
                    # q, k feature-major (k scaled by 1/sqrt(dhead))
                    for c in range(2 * DTILES):
                        wt = wsm.tile([P, KTILES, P], DT_R, name="wsm_t")
                        nc.sync.dma_start(out=wt, in_=wq3[:, :, bass.ts(c, P)])
                        pq = psm.tile([P, 512], DT_F, name="pmain")
                        for kt in range(KTILES):
                            nc.tensor.matmul(pq, wt[:, kt, :], h[:, kt, tsl],
                                             start=(kt == 0), stop=(kt == 7))
                        if c < DTILES:
                            nc.vector.tensor_copy(qf[:, c, :], pq)
                        else:
                            nc.vector.tensor_scalar(
                                kf[:, c - DTILES, :], pq,
                                float(DHEAD) ** -0.5, None, OP.mult)
                    # v token-major with ones column per head
                    for nh in range(2):
                        pvs = [psm.tile([P, 512], DT_F, name="pmain")
                               for _ in range(SEQT)]
                        for kt in range(KTILES):
                            wv = wsm.tile([P, 512], DT_R, name="wsm_t")
                            nc.sync.dma_start(
                                out=wv,
                                in_=wq3[:, kt, 2 * INNER + nh * 512:
                                        2 * INNER + (nh + 1) * 512])
                            for tt in range(SEQT):
                                nc.tensor.matmul(
                                    pvs[tt],
                                    h[:, kt, b * 512 + tt * P:
                                      b * 512 + (tt + 1) * P],
                                    wv,
                                    start=(kt == 0), stop=(kt == 7))
                        for tt in range(SEQT):
                            nc.vector.tensor_copy(
                                vt[:, tt, nh * 512:(nh + 1) * 512], pvs[tt])

                    for hd in range(HEADS):
                        po = (hd % 2) * 64
                        dt = hd // 2
                        sc = [psm.tile([P, 512], DT_F, name="pmain")
                              for _ in range(SEQT)]
                        # causal: j-tile jt only attends to i >= jt*P
                        for jt in range(SEQT):
                            i0 = jt * P
                            nc.tensor.matmul(
                                sc[jt][:, i0:],
                                kf[po:po + 64, dt, bass.ts(jt, P)],
                                qf[po:po + 64, dt, i0:],
                                start=True, stop=True)
                        etr = sm2.tile([P, SEQT, 512], DT_H, name="etr")
                        for jt in range(SEQT):
                            i0 = jt * P
                            ex = sm2.tile([P, 512], DT_F, name="ex")
                            nc.scalar.activation(ex[:, i0:], sc[jt][:, i0:],
                                                 FX.Exp)
                            nc.vector.tensor_mul(
                                etr[:, jt, i0:], ex[:, i0:],
                                et_sb[:, jt, i0:])
                        ot = pso.tile([P, 512], DT_F, name="ot")
                        den = pst.tile([1, 512], DT_F, name="ps0")
                        for jt in range(SEQT):
                            i0 = jt * P
                            nc.tensor.matmul(
                                ot[0:64, i0:],
                                vt[:, jt, hd * 64:(hd + 1) * 64],
                                etr[:, jt, i0:],
                                start=(jt == 0), stop=(jt == SEQT - 1))
                            nc.tensor.matmul(
                                den[:, i0:], ones_h, etr[:, jt, i0:],
                                start=(jt == 0), stop=(jt == SEQT - 1))
                        adr1 = stats.tile([1, 512], DT_F, name="adr1")
                        nc.vector.reciprocal(adr1, den)
                        adrb = stats.tile([64, 512], DT_F, name="adrb")
                        nc.gpsimd.partition_broadcast(adrb, adr1)
                        if po == 0:
                            nc.vector.tensor_mul(
                                of[0:64, dt, tsl], ot[0:64, :], adrb)
                        else:
                            # DVE can't shift partitions: normalize at base 0,
                            # DMA-copy up to partitions 64:128.
                            otmp = sm2.tile([64, 512], DT_R, name="otmp")
                            nc.vector.tensor_mul(otmp, ot[0:64, :], adrb)
                            nc.sync.dma_start(
                                out=of[64:128, dt, tsl], in_=otmp)

            wo3 = par["wo"][l].rearrange("(ko p) m -> p ko m", p=P)
            for dt in range(DTILES):
                wt = wsm.tile([P, KTILES, P], DT_R, name="wsm_t")
                nc.sync.dma_start(out=wt, in_=wo3[:, :, bass.ts(dt, P)])
                for b in range(BPC):
                    pq = psm.tile([P, 512], DT_F, name="pmain")
                    for kt in range(KTILES):
                        nc.tensor.matmul(
                            pq, wt[:, kt, :], of[:, kt, bass.ts(b, 512)],
                            start=(kt == 0), stop=(kt == 7))
                    nc.scalar.activation(pq, pq, FX.Identity,
                                         bias=bov_t[:, l, dt:dt + 1])
                    nc.vector.tensor_add(
                        x[:, dt, bass.ts(b, 512)], pq,
                        x[:, dt, bass.ts(b, 512)])

            h2 = big.tile([P, DTILES, TOK], DT_R, name="bigbuf")
            layer_norm_fm(lambda d: x[:, d, :], lambda d: h2[:, d, :],
                          DTILES, DIM, TOK,
                          lambda d: lnfg_t[:, l, d:d + 1],
                          lambda d: lnfb_t[:, l, d:d + 1])
            w13 = par["w1"][l].rearrange("(ko p) m -> p ko m", p=P)
            w23 = par["w2"][l].rearrange("(ko p) m -> p ko m", p=P)
            for th in range(2):
                tsl = bass.ts(th, 512)
                with tc.tile_pool(name=f"h1p{l}_{th}", bufs=1) as h1p:
                    h1r = h1p.tile([P, MTILES, 512], DT_H, name="h1r")
                    for mt in range(MTILES):
                        wt = wsm.tile([P, KTILES, P], DT_R, name="wsm_t")
                        nc.sync.dma_start(out=wt,
                                          in_=w13[:, :, bass.ts(mt, P)])
                        pq = psm.tile([P, 512], DT_F, name="pmain")
                        for kt in range(KTILES):
                            nc.tensor.matmul(pq, wt[:, kt, :], h2[:, kt, tsl],
                                             start=(kt == 0), stop=(kt == 7))
                        nc.scalar.activation(
                            h1r[:, mt, :], pq, FX.Gelu,
                            bias=b1v_t[:, l, mt:mt + 1], scale=1.0)
                    for dt in range(DTILES):
                        pq = psm.tile([P, 512], DT_F, name="pmain")
                        for kh in range(2):
                            wt = wsm.tile([P, 16, P], DT_H, name="wsm_t")
                            nc.sync.dma_start(
                                out=wt,
                                in_=w23[:, kh * 16:(kh + 1) * 16,
                                        bass.ts(dt, P)])
                            for k2 in range(16):
                                kt = kh * 16 + k2
                                nc.tensor.matmul(
                                    pq, wt[:, k2, :], h1r[:, kt, :],
                                    start=(kt == 0), stop=(kt == 31))
                        nc.scalar.activation(pq, pq, FX.Identity,
                                             bias=b2v_t[:, l, dt:dt + 1])
                        nc.vector.tensor_add(
                            x[:, dt, tsl], pq, x[:, dt, tsl])

        # =================== head ===================
        ho = big.tile([P, DTILES, TOK], DT_R, name="bigbuf")
        layer_norm_fm(lambda d: x[:, d, :], lambda d: ho[:, d, :],
                      DTILES, DIM, TOK,
                      lambda d: lnog_t[:, d:d + 1],
                      lambda d: lnob_t[:, d:d + 1])
        wp3 = par["wproj"].rearrange("(ko p) m -> p ko m", p=P)
        wt = wsm.tile([P, KTILES, NCLS], DT_R, name="wsm_t")
        nc.sync.dma_start(out=wt, in_=wp3)
        out_sb = one1.tile([NCLS, TOK], DT_F, name="out_sb")
        for th in range(2):
            pq = pso.tile([P, 512], DT_F, name="ot")
            for kt in range(KTILES):
                nc.tensor.matmul(pq[0:NCLS, :], wt[:, kt, :],
                                 ho[:, kt, bass.ts(th, 512)],
                                 start=(kt == 0), stop=(kt == 7))
            nc.scalar.activation(out_sb[:, bass.ts(th, 512)], pq[0:NCLS, :],
                                 FX.Identity, bias=bproj_t)
        nc.sync.dma_start(out=par["out"][:, :], in_=out_sb)


# ============================================================
# host side
# ============================================================

_NC_CACHE = None


def _host_band():
    tt = np.arange(KSIZE, dtype=np.float64)
    kern = np.exp(-0.5 * ((tt - (KSIZE - 1) / 2.0) / SIGMA) ** 2)
    kern = (kern / kern.sum()).astype(np.float32)
    pad_l = (KSIZE - 1) // 2  # 9
    nt = T // P
    bandc = np.zeros((nt, 3, P, P), dtype=np.float32)
    for ct in range(nt):
        for s in range(3):
            kt = ct - 1 + s
            if not (0 <= kt < nt):
                continue
            rows = np.arange(kt * P, (kt + 1) * P)
            cols = np.arange(ct * P, (ct + 1) * P)
            d = rows[:, None] - cols[None, :] + pad_l
            m = (d >= 0) & (d < KSIZE)
            blk = np.zeros((P, P), np.float32)
            blk[m] = kern[d[m]]
            bandc[ct, s] = blk
    return bandc


def _host_etab(rel_tab):
    i = np.arange(SEQ)
    j = i[:, None]
    rel = np.clip(i[None, :] - j, -(MAXREL - 1), MAXREL - 1) + MAXREL - 1
    et = np.zeros((DEPTH, SEQ, SEQ), dtype=np.float32)
    for l in range(DEPTH):
        e = np.exp(rel_tab[l][rel])
        e[j > i[None, :]] = 0.0
        et[l] = e
    return et.reshape(DEPTH, SEQT, P, SEQ)


def kernel(**inputs):
    global _NC_CACHE
    if _NC_CACHE is None:
        _NC_CACHE = build_nc()
    nc = _NC_CACHE

    f32 = lambda a: np.ascontiguousarray(np.asarray(a, dtype=np.float32))
    import ml_dtypes
    shared = {
        "band": _host_band(),
        "etab": _host_etab(f32(inputs["rel_tab"])),
        "wpe": f32(inputs["W_pe"]),
        "wqkv": f32(inputs["Wqkv"]),
        "wo": f32(inputs["Wo"]),
        "w1": f32(inputs["W1"]),
        "w2": f32(inputs["W2"]).astype(ml_dtypes.bfloat16),
        "wproj": f32(inputs["Wproj"]),
        "lnp1g": f32(inputs["ln_p1_g"]), "lnp1b": f32(inputs["ln_p1_b"]),
        "bpe": f32(inputs["b_pe"]),
        "lnp2g": f32(inputs["ln_p2_g"]), "lnp2b": f32(inputs["ln_p2_b"]),
        "lnag": f32(inputs["ln_a_g"]), "lnab": f32(inputs["ln_a_b"]),
        "lnfg": f32(inputs["ln_f_g"]), "lnfb": f32(inputs["ln_f_b"]),
        "bov": f32(inputs["bo"]), "b1v": f32(inputs["b1"]),
        "b2v": f32(inputs["b2"]),
        "lnog": f32(inputs["ln_o_g"]), "lnob": f32(inputs["ln_o_b"]),
        "bprojv": f32(inputs["bproj"]),
    }
    xfull = f32(inputs["neuralInput"])
    in_maps = []
    for c in range(NCORES):
        m = dict(shared)
        m["xin"] = np.ascontiguousarray(xfull[c * BPC:(c + 1) * BPC])
        in_maps.append(m)

    import os
    trace = bool(os.environ.get("BIT_TRACE"))
    res = run_bass_kernel_spmd(nc, in_maps, list(range(NCORES)), trace=trace)
    if trace:
        globals()["LAST_RESULT"] = res
    outs = []
    for c in range(NCORES):
        o = res.results[c]["out"]              # [NCLS, TOK]
        o = o.reshape(NCLS, BPC, SEQ).transpose(1, 2, 0)
        outs.append(o)
    return np.concatenate(outs, axis=0).astype(np.float32)


# revision 28
# speedup vs baseline: 1.1580x; 1.0926x over previous
"""Trainium2 Bass kernel for nn_BiT_Phoneme (dense transformer).

Data-parallel: 16 batch elems / 8 cores = 2 per core; each core runs the
full network on its 2 sequences (1024 "tokens"). Activations are kept
feature-major ([dim on partitions, tokens on free]) so matmuls chain
without transposes. The gaussian time-smoothing + patchify are a banded
matmul with a host-precomputed band matrix; patch extraction is a pure
access-pattern view. LayerNorm stats use ones-vector matmuls (partition
reduction); mean/rstd are broadcast back across partitions with K=1
matmuls into PSUM. Softmax runs transposed (keys on partitions) as
exp(scores)*E with E = exp(rel_bias)*causal_mask precomputed on host
(causal j-tiles only compute the valid i ranges); denominators come
from ones-vector matmuls over the probs; normalization is reciprocal +
gpsimd partition broadcast (odd heads are placed via an SBUF->SBUF DMA
partition shift, since DVE cannot cross partitions). Matmuls use
float32r (full PE rate at N>=256, ~1.5e-4 per-matmul accuracy; raw
fp32 bits are accepted for fp32r weight params). Attention probs / V /
h1 / W2 use bf16.
"""

import numpy as np

import concourse.bass as bass
import concourse.mybir as mybir
import concourse.tile as tile
from concourse import bacc
from concourse.bass_utils import run_bass_kernel_spmd

B, T, F = 16, 2048, 256
PH = 4
PATCH = 1024
DIM = 1024
DEPTH = 6
HEADS, DHEAD = 16, 64
INNER = 1024
MLP = 4096
NCLS = 41
MAXREL = 200
KSIZE, SIGMA = 20, 2.0
EPS = 1e-5
SEQ = T // PH              # 512
NCORES = 8
BPC = B // NCORES          # 2
TOK = BPC * SEQ            # 1024
P = 128

DT_R = mybir.dt.float32r
DT_F = mybir.dt.float32
DT_H = mybir.dt.bfloat16
FX = mybir.ActivationFunctionType
OP = mybir.AluOpType

DTILES = DIM // P          # 8
KTILES = DIM // P          # 8
MTILES = MLP // P          # 32
SEQT = SEQ // P            # 4


def build_nc():
    nc = bacc.Bacc(None, target_bir_lowering=False)

    par = {}
    def dp(name, shape, dtype, is_out=False):
        par[name] = nc.declare_dram_parameter(name, list(shape), dtype, isOutput=is_out)
        return par[name]

    dp("xin", (BPC, T, F), DT_R)
    dp("band", (T // P, 3, P, P), DT_R)
    dp("etab", (DEPTH, SEQT, P, SEQ), DT_F)
    dp("wpe", (PATCH, DIM), DT_R)
    dp("wqkv", (DEPTH, DIM, 3 * INNER), DT_H)
    dp("wo", (DEPTH, INNER, DIM), DT_H)
    dp("w1", (DEPTH, DIM, MLP), DT_H)
    dp("w2", (DEPTH, MLP, DIM), DT_H)
    dp("wproj", (DIM, NCLS), DT_R)
    for nm, shp in [("lnp1g", (PATCH,)), ("lnp1b", (PATCH,)), ("bpe", (DIM,)),
                    ("lnp2g", (DIM,)), ("lnp2b", (DIM,)),
                    ("lnag", (DEPTH, DIM)), ("lnab", (DEPTH, DIM)),
                    ("lnfg", (DEPTH, DIM)), ("lnfb", (DEPTH, DIM)),
                    ("bov", (DEPTH, DIM)), ("b1v", (DEPTH, MLP)),
                    ("b2v", (DEPTH, DIM)), ("lnog", (DIM,)), ("lnob", (DIM,)),
                    ("bprojv", (NCLS,))]:
        dp(nm, shp, DT_F)
    dp("out", (NCLS, TOK), DT_F, is_out=True)

    with tile.TileContext(nc) as tc:
        _emit(nc, tc, par)
    nc.compile()
    return nc


def _emit(nc, tc, par):
    import contextlib
    ctx = contextlib.ExitStack()
    with ctx:
        const = ctx.enter_context(tc.tile_pool(name="const", bufs=1))
        xpool = ctx.enter_context(tc.tile_pool(name="xpool", bufs=1))
        big = ctx.enter_context(tc.tile_pool(name="big", bufs=2))
        wsm = ctx.enter_context(tc.tile_pool(name="wsm", bufs=3))
        stats = ctx.enter_context(tc.tile_pool(name="stats", bufs=1))
        sm2 = ctx.enter_context(tc.tile_pool(name="sm2", bufs=2))
        one1 = ctx.enter_context(tc.tile_pool(name="one1", bufs=1))
        psm = ctx.enter_context(tc.tile_pool(name="psm", bufs=4, space="PSUM"))
        pso = ctx.enter_context(tc.tile_pool(name="pso", bufs=2, space="PSUM"))
        pst = ctx.enter_context(tc.tile_pool(name="pst", bufs=1, space="PSUM"))

        ones_r = const.tile([P, 1], DT_R, name="ones_r")
        nc.vector.memset(ones_r.bitcast(mybir.dt.uint32), 0x3F800000)
        ones_h = const.tile([P, 1], DT_H, name="ones_h")
        nc.vector.memset(ones_h.bitcast(mybir.dt.uint16), 0x3F80)
        ones_row = const.tile([1, P], DT_R, name="ones_row")
        nc.vector.memset(ones_row.bitcast(mybir.dt.uint32), 0x3F800000)
        epst = const.tile([1, 1], DT_F, name="epst")
        nc.vector.memset(epst, EPS)

        def load_vec(nm, width):
            d = par[nm]
            if len(d.shape) == 1:
                tl = const.tile([P, width // P], DT_F, name=nm + "_t")
                nc.sync.dma_start(out=tl, in_=d.rearrange("(o p) -> p o", p=P))
            else:
                L = d.shape[0]
                tl = const.tile([P, L, width // P], DT_F, name=nm + "_t")
                nc.sync.dma_start(out=tl, in_=d.rearrange("l (o p) -> p l o", p=P))
            return tl

        lnp1g_t = load_vec("lnp1g", PATCH)
        lnp1b_t = load_vec("lnp1b", PATCH)
        bpe_t = load_vec("bpe", DIM)
        lnp2g_t = load_vec("lnp2g", DIM)
        lnp2b_t = load_vec("lnp2b", DIM)
        lnag_t = load_vec("lnag", DIM)
        lnab_t = load_vec("lnab", DIM)
        lnfg_t = load_vec("lnfg", DIM)
        lnfb_t = load_vec("lnfb", DIM)
        bov_t = load_vec("bov", DIM)
        b1v_t = load_vec("b1v", MLP)
        b2v_t = load_vec("b2v", DIM)
        lnog_t = load_vec("lnog", DIM)
        lnob_t = load_vec("lnob", DIM)
        bproj_t = const.tile([NCLS, 1], DT_F, name="bproj_t")
        nc.sync.dma_start(out=bproj_t,
                          in_=par["bprojv"].rearrange("(p o) -> p o", o=1))

        x = xpool.tile([P, DTILES, TOK], DT_R, name="x")

        # ---- layernorm over partitions+tiles (feature-major) ----
        # views(d) -> [P, width] fp32r source slices (ntiles of them);
        # writes dst_fn(d) slices, width tokens, nh 512-halves.
        def layer_norm_fm(views, dst_fn, ntiles, D, width, g_fn, b_fn):
            nh = width // 512
            st = stats.tile([1, 2, TOK], DT_R, name="st")
            mu, rstd = st[:, 0, :width], st[:, 1, :width]
            vt32 = stats.tile([1, TOK], DT_F, name="vt32")
            vtmp = vt32[:, :width]
            for th in range(nh):
                sl = bass.ts(th, 512)
                ps0 = pst.tile([1, 512], DT_F, name="ps0")
                ps1 = pst.tile([1, 512], DT_F, name="ps1")
                for d in range(ntiles):
                    v = views(d)[:, sl]
                    sq = sm2.tile([P, 512], DT_R, name="sq")
                    nc.vector.tensor_mul(sq, v, v)
                    nc.tensor.matmul(ps0, ones_r, v,
                                     start=(d == 0), stop=(d == ntiles - 1))
                    nc.tensor.matmul(ps1, ones_r, sq,
                                     start=(d == 0), stop=(d == ntiles - 1))
                nc.vector.tensor_scalar(mu[:, sl], ps0, 1.0 / D, None, OP.mult)
                # rstd slot <- mu^2 (scratch), vt32 <- E[x^2] - mu^2
                nc.vector.tensor_scalar(vtmp[:, sl], ps1, 1.0 / D, None,
                                        OP.mult)
                nc.vector.tensor_mul(rstd[:, sl], mu[:, sl], mu[:, sl])
                nc.vector.tensor_sub(vtmp[:, sl], vtmp[:, sl], rstd[:, sl])
            nc.scalar.activation(vtmp, vtmp, FX.Sqrt, bias=epst, scale=1.0)
            with nc.allow_low_precision(reason="fp32r rstd feeds matmul bcast"):
                nc.vector.reciprocal(rstd, vtmp)
            for th in range(nh):
                sl = bass.ts(th, 512)
                # broadcast mu/rstd across partitions via K=1 matmul
                mps = psm.tile([P, 512], DT_F, name="pmain")
                rps = psm.tile([P, 512], DT_F, name="pmain")
                nc.tensor.matmul(mps, ones_row, mu[:, sl],
                                 start=True, stop=True)
                nc.tensor.matmul(rps, ones_row, rstd[:, sl],
                                 start=True, stop=True)
                for d in range(ntiles):
                    tmp = sm2.tile([P, 512], DT_F, name="lntmp")
                    nc.vector.tensor_sub(tmp, views(d)[:, sl], mps)
                    nc.vector.tensor_mul(tmp, tmp, rps)
                    nc.vector.tensor_scalar(
                        dst_fn(d)[:, sl], tmp, g_fn(d), b_fn(d),
                        OP.mult, OP.add)

        # =================== embedding ===================
        xin, band = par["xin"], par["band"]
        with (
            tc.tile_pool(name="sfp", bufs=1) as sfp,
            tc.tile_pool(name="pnp", bufs=1) as pnp,
        ):
            x0 = big.tile([P, DTILES, TOK], DT_R, name="bigbuf")
            for b in range(BPC):
                sf = sfp.tile([P, 2, T], DT_R, name="sf")
                for fh in range(2):
                    for g4 in range(T // 512):
                        pg = psm.tile([P, 512], DT_F, name="pmain")
                        for q in range(4):
                            ct = g4 * 4 + q
                            bt = wsm.tile([P, 3, P], DT_R, name="wsm_t")
                            nc.sync.dma_start(
                                out=bt,
                                in_=band[ct].rearrange("s p q -> p s q"))
                            svals = [s for s in range(3)
                                     if 0 <= ct - 1 + s < T // P]
                            for si, s in enumerate(svals):
                                kt = ct - 1 + s
                                xa = wsm.tile([P, P], DT_R, name="wsm_t")
                                nc.sync.dma_start(
                                    out=xa,
                                    in_=xin[b, bass.ts(kt, P),
                                            bass.ts(fh, P)])
                                nc.tensor.matmul(
                                    pg[:, bass.ts(q, P)],
                                    xa,
                                    bt[:, s, :],
                                    start=(si == 0),
                                    stop=(si == len(svals) - 1))
                        nc.vector.tensor_copy(sf[:, fh, bass.ts(g4, 512)], pg)

                def pview(pt):
                    i, fh = pt // 2, pt % 2
                    return sf[:, fh, :].rearrange(
                        "p (s four) -> p four s", four=PH)[:, i, :]

                pn = pnp.tile([P, 8, 512], DT_R, name="pn")
                layer_norm_fm(pview, lambda d: pn[:, d, :], 8, PATCH, 512,
                              lambda d: lnp1g_t[:, d:d + 1],
                              lambda d: lnp1b_t[:, d:d + 1])

                for dt in range(DTILES):
                    wt = wsm.tile([P, KTILES, P], DT_R, name="wsm_t")
                    nc.sync.dma_start(
                        out=wt,
                        in_=par["wpe"].rearrange("(ko p) m -> p ko m", p=P)[
                            :, :, bass.ts(dt, P)])
                    pq = psm.tile([P, 512], DT_F, name="pmain")
                    for kt in range(KTILES):
                        nc.tensor.matmul(pq, wt[:, kt, :], pn[:, kt, :],
                                         start=(kt == 0), stop=(kt == 7))
                    nc.vector.tensor_scalar(
                        x0[:, dt, bass.ts(b, 512)], pq,
                        bpe_t[:, dt:dt + 1], None, OP.add)

            layer_norm_fm(lambda d: x0[:, d, :], lambda d: x[:, d, :],
                          DTILES, DIM, TOK,
                          lambda d: lnp2g_t[:, d:d + 1],
                          lambda d: lnp2b_t[:, d:d + 1])

        # =================== transformer layers ===================
        for l in range(DEPTH):
            h = big.tile([P, DTILES, TOK], DT_H, name="bigbuf")
            with nc.named_scope(f"L{l}_lna"):
                layer_norm_fm(lambda d: x[:, d, :], lambda d: h[:, d, :],
                              DTILES, DIM, TOK,
                              lambda d: lnag_t[:, l, d:d + 1],
                              lambda d: lnab_t[:, l, d:d + 1])

            et_sb = one1.tile([P, SEQT, SEQ], DT_F, name="et_sb")
            nc.sync.dma_start(out=et_sb,
                              in_=par["etab"][l].rearrange("jt p i -> p jt i"))

            of = big.tile([P, DTILES, TOK], DT_H, name="bigbuf")
            wq3 = par["wqkv"][l].rearrange("(ko p) m -> p ko m", p=P)
            for b in range(BPC):
                tsl = bass.ts(b, 512)
                with tc.tile_pool(name=f"qkv{l}_{b}", bufs=1) as qkvp:
                    qf = qkvp.tile([P, DTILES, 512], DT_H, name="qf")
                    kf = qkvp.tile([P, DTILES, 512], DT_H, name="kf")
                    vt = qkvp.tile([P, SEQT, INNER], DT_H, name="vt")
                    # q, k feature-major (k scaled by 1/sqrt(dhead))
                    for c in range(2 * DTILES):
                        wt = wsm.tile([P, KTILES, P], DT_H, name="wsm_t")
                        nc.sync.dma_start(out=wt, in_=wq3[:, :, bass.ts(c, P)])
                        pq = psm.tile([P, 512], DT_F, name="pmain")
                        for kt in range(KTILES):
                            nc.tensor.matmul(pq, wt[:, kt, :], h[:, kt, tsl],
                                             start=(kt == 0), stop=(kt == 7))
                        if c < DTILES:
                            nc.vector.tensor_copy(qf[:, c, :], pq)
                        else:
                            nc.vector.tensor_scalar(
                                kf[:, c - DTILES, :], pq,
                                float(DHEAD) ** -0.5, None, OP.mult)
                    # v token-major with ones column per head
                    for nh in range(2):
                        pvs = [psm.tile([P, 512], DT_F, name="pmain")
                               for _ in range(SEQT)]
                        for kt in range(KTILES):
                            wv = wsm.tile([P, 512], DT_H, name="wsm_t")
                            nc.sync.dma_start(
                                out=wv,
                                in_=wq3[:, kt, 2 * INNER + nh * 512:
                                        2 * INNER + (nh + 1) * 512])
                            for tt in range(SEQT):
                                nc.tensor.matmul(
                                    pvs[tt],
                                    h[:, kt, b * 512 + tt * P:
                                      b * 512 + (tt + 1) * P],
                                    wv,
                                    start=(kt == 0), stop=(kt == 7))
                        for tt in range(SEQT):
                            nc.vector.tensor_copy(
                                vt[:, tt, nh * 512:(nh + 1) * 512], pvs[tt])

                    for hd in range(HEADS):
                        po = (hd % 2) * 64
                        dt = hd // 2
                        sc = [psm.tile([P, 512], DT_F, name="pmain")
                              for _ in range(SEQT)]
                        # causal: j-tile jt only attends to i >= jt*P
                        for jt in range(SEQT):
                            i0 = jt * P
                            nc.tensor.matmul(
                                sc[jt][:, i0:],
                                kf[po:po + 64, dt, bass.ts(jt, P)],
                                qf[po:po + 64, dt, i0:],
                                start=True, stop=True)
                        etr = sm2.tile([P, SEQT, 512], DT_H, name="etr")
                        for jt in range(SEQT):
                            i0 = jt * P
                            ex = sm2.tile([P, 512], DT_F, name="ex")
                            nc.scalar.activation(ex[:, i0:], sc[jt][:, i0:],
                                                 FX.Exp)
                            nc.vector.tensor_mul(
                                etr[:, jt, i0:], ex[:, i0:],
                                et_sb[:, jt, i0:])
                        ot = pso.tile([P, 512], DT_F, name="ot")
                        den = pst.tile([1, 512], DT_F, name="ps0")
                        for jt in range(SEQT):
                            i0 = jt * P
                            nc.tensor.matmul(
                                ot[0:64, i0:],
                                vt[:, jt, hd * 64:(hd + 1) * 64],
                                etr[:, jt, i0:],
                                start=(jt == 0), stop=(jt == SEQT - 1))
                            nc.tensor.matmul(
                                den[:, i0:], ones_h, etr[:, jt, i0:],
                                start=(jt == 0), stop=(jt == SEQT - 1))
                        adr1 = stats.tile([1, 512], DT_F, name="adr1")
                        nc.vector.reciprocal(adr1, den)
                        adrb = stats.tile([64, 512], DT_F, name="adrb")
                        nc.gpsimd.partition_broadcast(adrb, adr1)
                        if po == 0:
                            nc.vector.tensor_mul(
                                of[0:64, dt, tsl], ot[0:64, :], adrb)
                        else:
                            # DVE can't shift partitions: normalize at base 0,
                            # DMA-copy up to partitions 64:128.
                            otmp = sm2.tile([64, 512], DT_H, name="otmp")
                            nc.vector.tensor_mul(otmp, ot[0:64, :], adrb)
                            nc.sync.dma_start(
                                out=of[64:128, dt, tsl], in_=otmp)

            wo3 = par["wo"][l].rearrange("(ko p) m -> p ko m", p=P)
            for dt in range(DTILES):
                wt = wsm.tile([P, KTILES, P], DT_H, name="wsm_t")
                nc.sync.dma_start(out=wt, in_=wo3[:, :, bass.ts(dt, P)])
                for b in range(BPC):
                    pq = psm.tile([P, 512], DT_F, name="pmain")
                    for kt in range(KTILES):
                        nc.tensor.matmul(
                            pq, wt[:, kt, :], of[:, kt, bass.ts(b, 512)],
                            start=(kt == 0), stop=(kt == 7))
                    nc.scalar.activation(pq, pq, FX.Identity,
                                         bias=bov_t[:, l, dt:dt + 1])
                    nc.vector.tensor_add(
                        x[:, dt, bass.ts(b, 512)], pq,
                        x[:, dt, bass.ts(b, 512)])

            h2 = big.tile([P, DTILES, TOK], DT_H, name="bigbuf")
            layer_norm_fm(lambda d: x[:, d, :], lambda d: h2[:, d, :],
                          DTILES, DIM, TOK,
                          lambda d: lnfg_t[:, l, d:d + 1],
                          lambda d: lnfb_t[:, l, d:d + 1])
            w13 = par["w1"][l].rearrange("(ko p) m -> p ko m", p=P)
            w23 = par["w2"][l].rearrange("(ko p) m -> p ko m", p=P)
            for th in range(2):
                tsl = bass.ts(th, 512)
                with tc.tile_pool(name=f"h1p{l}_{th}", bufs=1) as h1p:
                    h1r = h1p.tile([P, MTILES, 512], DT_H, name="h1r")
                    for mt in range(MTILES):
                        wt = wsm.tile([P, KTILES, P], DT_H, name="wsm_t")
                        nc.sync.dma_start(out=wt,
                                          in_=w13[:, :, bass.ts(mt, P)])
                        pq = psm.tile([P, 512], DT_F, name="pmain")
                        for kt in range(KTILES):
                            nc.tensor.matmul(pq, wt[:, kt, :], h2[:, kt, tsl],
                                             start=(kt == 0), stop=(kt == 7))
                        nc.scalar.activation(
                            h1r[:, mt, :], pq, FX.Gelu,
                            bias=b1v_t[:, l, mt:mt + 1], scale=1.0)
                    for dt in range(DTILES):
                        pq = psm.tile([P, 512], DT_F, name="pmain")
                        for kh in range(2):
                            wt = wsm.tile([P, 16, P], DT_H, name="wsm_t")
                            nc.sync.dma_start(
                                out=wt,
                                in_=w23[:, kh * 16:(kh + 1) * 16,
                                        bass.ts(dt, P)])
                            for k2 in range(16):
                                kt = kh * 16 + k2
                                nc.tensor.matmul(
                                    pq, wt[:, k2, :], h1r[:, kt, :],
                                    start=(kt == 0), stop=(kt == 31))
                        nc.scalar.activation(pq, pq, FX.Identity,
                                             bias=b2v_t[:, l, dt:dt + 1])
                        nc.vector.tensor_add(
                            x[:, dt, tsl], pq, x[:, dt, tsl])

        # =================== head ===================
        ho = big.tile([P, DTILES, TOK], DT_R, name="bigbuf")
        layer_norm_fm(lambda d: x[:, d, :], lambda d: ho[:, d, :],
                      DTILES, DIM, TOK,
                      lambda d: lnog_t[:, d:d + 1],
                      lambda d: lnob_t[:, d:d + 1])
        wp3 = par["wproj"].rearrange("(ko p) m -> p ko m", p=P)
        wt = wsm.tile([P, KTILES, NCLS], DT_R, name="wsm_t")
        nc.sync.dma_start(out=wt, in_=wp3)
        out_sb = one1.tile([NCLS, TOK], DT_F, name="out_sb")
        for th in range(2):
            pq = pso.tile([P, 512], DT_F, name="ot")
            for kt in range(KTILES):
                nc.tensor.matmul(pq[0:NCLS, :], wt[:, kt, :],
                                 ho[:, kt, bass.ts(th, 512)],
                                 start=(kt == 0), stop=(kt == 7))
            nc.scalar.activation(out_sb[:, bass.ts(th, 512)], pq[0:NCLS, :],
                                 FX.Identity, bias=bproj_t)
        nc.sync.dma_start(out=par["out"][:, :], in_=out_sb)


# ============================================================
# host side
# ============================================================

_NC_CACHE = None


def _host_band():
    tt = np.arange(KSIZE, dtype=np.float64)
    kern = np.exp(-0.5 * ((tt - (KSIZE - 1) / 2.0) / SIGMA) ** 2)
    kern = (kern / kern.sum()).astype(np.float32)
    pad_l = (KSIZE - 1) // 2  # 9
    nt = T // P
    bandc = np.zeros((nt, 3, P, P), dtype=np.float32)
    for ct in range(nt):
        for s in range(3):
            kt = ct - 1 + s
            if not (0 <= kt < nt):
                continue
            rows = np.arange(kt * P, (kt + 1) * P)
            cols = np.arange(ct * P, (ct + 1) * P)
            d = rows[:, None] - cols[None, :] + pad_l
            m = (d >= 0) & (d < KSIZE)
            blk = np.zeros((P, P), np.float32)
            blk[m] = kern[d[m]]
            bandc[ct, s] = blk
    return bandc


def _host_etab(rel_tab):
    i = np.arange(SEQ)
    j = i[:, None]
    rel = np.clip(i[None, :] - j, -(MAXREL - 1), MAXREL - 1) + MAXREL - 1
    et = np.zeros((DEPTH, SEQ, SEQ), dtype=np.float32)
    for l in range(DEPTH):
        e = np.exp(rel_tab[l][rel])
        e[j > i[None, :]] = 0.0
        et[l] = e
    return et.reshape(DEPTH, SEQT, P, SEQ)


def kernel(**inputs):
    global _NC_CACHE
    if _NC_CACHE is None:
        _NC_CACHE = build_nc()
    nc = _NC_CACHE

    f32 = lambda a: np.ascontiguousarray(np.asarray(a, dtype=np.float32))
    import ml_dtypes
    shared = {
        "band": _host_band(),
        "etab": _host_etab(f32(inputs["rel_tab"])),
        "wpe": f32(inputs["W_pe"]),
        "wqkv": f32(inputs["Wqkv"]).astype(ml_dtypes.bfloat16),
        "wo": f32(inputs["Wo"]).astype(ml_dtypes.bfloat16),
        "w1": f32(inputs["W1"]).astype(ml_dtypes.bfloat16),
        "w2": f32(inputs["W2"]).astype(ml_dtypes.bfloat16),
        "wproj": f32(inputs["Wproj"]),
        "lnp1g": f32(inputs["ln_p1_g"]), "lnp1b": f32(inputs["ln_p1_b"]),
        "bpe": f32(inputs["b_pe"]),
        "lnp2g": f32(inputs["ln_p2_g"]), "lnp2b": f32(inputs["ln_p2_b"]),
        "lnag": f32(inputs["ln_a_g"]), "lnab": f32(inputs["ln_a_b"]),
        "lnfg": f32(inputs["ln_f_g"]), "lnfb": f32(inputs["ln_f_b"]),
        "bov": f32(inputs["bo"]), "b1v": f32(inputs["b1"]),
        "b2v": f32(inputs["b2"]),
        "lnog": f32(inputs["ln_o_g"]), "lnob": f32(inputs["ln_o_b"]),
        "bprojv": f32(inputs["bproj"]),
    }
    xfull = f32(inputs["neuralInput"])
    in_maps = []
    for c in range(NCORES):
        m = dict(shared)
        m["xin"] = np.ascontiguousarray(xfull[c * BPC:(c + 1) * BPC])
        in_maps.append(m)

    import os
    trace = bool(os.environ.get("BIT_TRACE"))
    res = run_bass_kernel_spmd(nc, in_maps, list(range(NCORES)), trace=trace)
    if trace:
        globals()["LAST_RESULT"] = res
    outs = []
    for c in range(NCORES):
        o = res.results[c]["out"]              # [NCLS, TOK]
        o = o.reshape(NCLS, BPC, SEQ).transpose(1, 2, 0)
        outs.append(o)
    return np.concatenate(outs, axis=0).astype(np.float32)


# revision 29
# speedup vs baseline: 1.3430x; 1.1598x over previous
"""Trainium2 Bass kernel for nn_BiT_Phoneme (dense transformer).

Data-parallel: 16 batch elems / 8 cores = 2 per core; each core runs the
full network on its 2 sequences (1024 "tokens"). Activations are kept
feature-major ([dim on partitions, tokens on free]) so matmuls chain
without transposes. The gaussian time-smoothing + patchify are a banded
matmul with a host-precomputed band matrix; patch extraction is a pure
access-pattern view. LayerNorm stats use ones-vector matmuls (partition
reduction); mean/rstd are broadcast back across partitions with K=1
matmuls into PSUM. Softmax runs transposed (keys on partitions) as
exp(scores)*E with E = exp(rel_bias)*causal_mask precomputed on host
(causal j-tiles only compute the valid i ranges); denominators come
from ones-vector matmuls over the probs; normalization is reciprocal +
gpsimd partition broadcast (odd heads are placed via an SBUF->SBUF DMA
partition shift, since DVE cannot cross partitions). Matmuls use
float32r (full PE rate at N>=256, ~1.5e-4 per-matmul accuracy; raw
fp32 bits are accepted for fp32r weight params). Attention probs / V /
h1 / W2 use bf16.
"""

import numpy as np

import concourse.bass as bass
import concourse.mybir as mybir
import concourse.tile as tile
from concourse import bacc
from concourse.bass_utils import run_bass_kernel_spmd

B, T, F = 16, 2048, 256
PH = 4
PATCH = 1024
DIM = 1024
DEPTH = 6
HEADS, DHEAD = 16, 64
INNER = 1024
MLP = 4096
NCLS = 41
MAXREL = 200
KSIZE, SIGMA = 20, 2.0
EPS = 1e-5
SEQ = T // PH              # 512
NCORES = 8
BPC = B // NCORES          # 2
TOK = BPC * SEQ            # 1024
P = 128

DT_R = mybir.dt.float32r
DT_F = mybir.dt.float32
DT_H = mybir.dt.bfloat16
FX = mybir.ActivationFunctionType
OP = mybir.AluOpType

DTILES = DIM // P          # 8
KTILES = DIM // P          # 8
MTILES = MLP // P          # 32
SEQT = SEQ // P            # 4


def build_nc():
    nc = bacc.Bacc(None, target_bir_lowering=False)

    par = {}
    def dp(name, shape, dtype, is_out=False):
        par[name] = nc.declare_dram_parameter(name, list(shape), dtype, isOutput=is_out)
        return par[name]

    dp("xin", (BPC, T, F), DT_R)
    dp("band", (T // P, 3, P, P), DT_R)
    dp("etab", (DEPTH, SEQT, P, SEQ), DT_F)
    dp("wpe", (PATCH, DIM), DT_R)
    dp("wqkv", (DEPTH, DIM, 3 * INNER), DT_H)
    dp("wo", (DEPTH, INNER, DIM), DT_H)
    dp("w1", (DEPTH, DIM, MLP), DT_H)
    dp("w2", (DEPTH, MLP, DIM), DT_H)
    dp("wproj", (DIM, NCLS), DT_R)
    for nm, shp in [("lnp1g", (PATCH,)), ("lnp1b", (PATCH,)), ("bpe", (DIM,)),
                    ("lnp2g", (DIM,)), ("lnp2b", (DIM,)),
                    ("lnag", (DEPTH, DIM)), ("lnab", (DEPTH, DIM)),
                    ("lnfg", (DEPTH, DIM)), ("lnfb", (DEPTH, DIM)),
                    ("bov", (DEPTH, DIM)), ("b1v", (DEPTH, MLP)),
                    ("b2v", (DEPTH, DIM)), ("lnog", (DIM,)), ("lnob", (DIM,)),
                    ("bprojv", (NCLS,))]:
        dp(nm, shp, DT_F)
    dp("out", (NCLS, TOK), DT_F, is_out=True)

    with tile.TileContext(nc) as tc:
        _emit(nc, tc, par)
    nc.compile()
    return nc


def _emit(nc, tc, par):
    import contextlib
    ctx = contextlib.ExitStack()
    with ctx:
        const = ctx.enter_context(tc.tile_pool(name="const", bufs=1))
        xpool = ctx.enter_context(tc.tile_pool(name="xpool", bufs=1))
        big = ctx.enter_context(tc.tile_pool(name="big", bufs=2))
        wsm = ctx.enter_context(tc.tile_pool(name="wsm", bufs=4))
        stats = ctx.enter_context(tc.tile_pool(name="stats", bufs=1))
        sm2 = ctx.enter_context(tc.tile_pool(name="sm2", bufs=2))
        one1 = ctx.enter_context(tc.tile_pool(name="one1", bufs=1))
        psm = ctx.enter_context(tc.tile_pool(name="psm", bufs=4, space="PSUM"))
        pso = ctx.enter_context(tc.tile_pool(name="pso", bufs=2, space="PSUM"))
        pst = ctx.enter_context(tc.tile_pool(name="pst", bufs=1, space="PSUM"))

        ones_r = const.tile([P, 1], DT_R, name="ones_r")
        nc.vector.memset(ones_r.bitcast(mybir.dt.uint32), 0x3F800000)
        ones_h = const.tile([P, 1], DT_H, name="ones_h")
        nc.vector.memset(ones_h.bitcast(mybir.dt.uint16), 0x3F80)
        ones_row = const.tile([1, P], DT_R, name="ones_row")
        nc.vector.memset(ones_row.bitcast(mybir.dt.uint32), 0x3F800000)
        epst = const.tile([1, 1], DT_F, name="epst")
        nc.vector.memset(epst, EPS)

        def load_vec(nm, width):
            d = par[nm]
            if len(d.shape) == 1:
                tl = const.tile([P, width // P], DT_F, name=nm + "_t")
                nc.sync.dma_start(out=tl, in_=d.rearrange("(o p) -> p o", p=P))
            else:
                L = d.shape[0]
                tl = const.tile([P, L, width // P], DT_F, name=nm + "_t")
                nc.sync.dma_start(out=tl, in_=d.rearrange("l (o p) -> p l o", p=P))
            return tl

        lnp1g_t = load_vec("lnp1g", PATCH)
        lnp1b_t = load_vec("lnp1b", PATCH)
        bpe_t = load_vec("bpe", DIM)
        lnp2g_t = load_vec("lnp2g", DIM)
        lnp2b_t = load_vec("lnp2b", DIM)
        lnag_t = load_vec("lnag", DIM)
        lnab_t = load_vec("lnab", DIM)
        lnfg_t = load_vec("lnfg", DIM)
        lnfb_t = load_vec("lnfb", DIM)
        bov_t = load_vec("bov", DIM)
        b1v_t = load_vec("b1v", MLP)
        b2v_t = load_vec("b2v", DIM)
        lnog_t = load_vec("lnog", DIM)
        lnob_t = load_vec("lnob", DIM)
        bproj_t = const.tile([NCLS, 1], DT_F, name="bproj_t")
        nc.sync.dma_start(out=bproj_t,
                          in_=par["bprojv"].rearrange("(p o) -> p o", o=1))

        x = xpool.tile([P, DTILES, TOK], DT_R, name="x")

        # ---- layernorm over partitions+tiles (feature-major) ----
        # views(d) -> [P, width] fp32r source slices (ntiles of them);
        # writes dst_fn(d) slices, width tokens, nh 512-halves.
        def layer_norm_fm(views, dst_fn, ntiles, D, width, g_fn, b_fn):
            nh = width // 512
            st = stats.tile([1, 2, TOK], DT_R, name="st")
            mu, rstd = st[:, 0, :width], st[:, 1, :width]
            vt32 = stats.tile([1, TOK], DT_F, name="vt32")
            vtmp = vt32[:, :width]
            for th in range(nh):
                sl = bass.ts(th, 512)
                ps0 = pst.tile([1, 512], DT_F, name="ps0")
                ps1 = pst.tile([1, 512], DT_F, name="ps1")
                for d in range(ntiles):
                    v = views(d)[:, sl]
                    sq = sm2.tile([P, 512], DT_R, name="sq")
                    nc.vector.tensor_mul(sq, v, v)
                    nc.tensor.matmul(ps0, ones_r, v,
                                     start=(d == 0), stop=(d == ntiles - 1))
                    nc.tensor.matmul(ps1, ones_r, sq,
                                     start=(d == 0), stop=(d == ntiles - 1))
                nc.vector.tensor_scalar(mu[:, sl], ps0, 1.0 / D, None, OP.mult)
                # rstd slot <- mu^2 (scratch), vt32 <- E[x^2] - mu^2
                nc.vector.tensor_scalar(vtmp[:, sl], ps1, 1.0 / D, None,
                                        OP.mult)
                nc.vector.tensor_mul(rstd[:, sl], mu[:, sl], mu[:, sl])
                nc.vector.tensor_sub(vtmp[:, sl], vtmp[:, sl], rstd[:, sl])
            nc.scalar.activation(vtmp, vtmp, FX.Sqrt, bias=epst, scale=1.0)
            with nc.allow_low_precision(reason="fp32r rstd feeds matmul bcast"):
                nc.vector.reciprocal(rstd, vtmp)
            for th in range(nh):
                sl = bass.ts(th, 512)
                # broadcast mu/rstd across partitions via K=1 matmul
                mps = psm.tile([P, 512], DT_F, name="pmain")
                rps = psm.tile([P, 512], DT_F, name="pmain")
                nc.tensor.matmul(mps, ones_row, mu[:, sl],
                                 start=True, stop=True)
                nc.tensor.matmul(rps, ones_row, rstd[:, sl],
                                 start=True, stop=True)
                for d in range(ntiles):
                    tmp = sm2.tile([P, 512], DT_F, name="lntmp")
                    nc.vector.tensor_sub(tmp, views(d)[:, sl], mps)
                    nc.vector.tensor_mul(tmp, tmp, rps)
                    nc.vector.tensor_scalar(
                        dst_fn(d)[:, sl], tmp, g_fn(d), b_fn(d),
                        OP.mult, OP.add)

        # =================== embedding ===================
        xin, band = par["xin"], par["band"]
        with (
            tc.tile_pool(name="sfp", bufs=1) as sfp,
            tc.tile_pool(name="pnp", bufs=1) as pnp,
        ):
            x0 = big.tile([P, DTILES, TOK], DT_R, name="bigbuf")
            for b in range(BPC):
                sf = sfp.tile([P, 2, T], DT_R, name="sf")
                for fh in range(2):
                    for g4 in range(T // 512):
                        pg = psm.tile([P, 512], DT_F, name="pmain")
                        for q in range(4):
                            ct = g4 * 4 + q
                            bt = wsm.tile([P, 3, P], DT_R, name="wsm_t")
                            nc.sync.dma_start(
                                out=bt,
                                in_=band[ct].rearrange("s p q -> p s q"))
                            svals = [s for s in range(3)
                                     if 0 <= ct - 1 + s < T // P]
                            for si, s in enumerate(svals):
                                kt = ct - 1 + s
                                xa = wsm.tile([P, P], DT_R, name="wsm_t")
                                nc.sync.dma_start(
                                    out=xa,
                                    in_=xin[b, bass.ts(kt, P),
                                            bass.ts(fh, P)])
                                nc.tensor.matmul(
                                    pg[:, bass.ts(q, P)],
                                    xa,
                                    bt[:, s, :],
                                    start=(si == 0),
                                    stop=(si == len(svals) - 1))
                        nc.vector.tensor_copy(sf[:, fh, bass.ts(g4, 512)], pg)

                def pview(pt):
                    i, fh = pt // 2, pt % 2
                    return sf[:, fh, :].rearrange(
                        "p (s four) -> p four s", four=PH)[:, i, :]

                pn = pnp.tile([P, 8, 512], DT_R, name="pn")
                layer_norm_fm(pview, lambda d: pn[:, d, :], 8, PATCH, 512,
                              lambda d: lnp1g_t[:, d:d + 1],
                              lambda d: lnp1b_t[:, d:d + 1])

                for dt in range(DTILES):
                    wt = wsm.tile([P, KTILES, P], DT_R, name="wsm_t")
                    nc.sync.dma_start(
                        out=wt,
                        in_=par["wpe"].rearrange("(ko p) m -> p ko m", p=P)[
                            :, :, bass.ts(dt, P)])
                    pq = psm.tile([P, 512], DT_F, name="pmain")
                    for kt in range(KTILES):
                        nc.tensor.matmul(pq, wt[:, kt, :], pn[:, kt, :],
                                         start=(kt == 0), stop=(kt == 7))
                    nc.vector.tensor_scalar(
                        x0[:, dt, bass.ts(b, 512)], pq,
                        bpe_t[:, dt:dt + 1], None, OP.add)

            layer_norm_fm(lambda d: x0[:, d, :], lambda d: x[:, d, :],
                          DTILES, DIM, TOK,
                          lambda d: lnp2g_t[:, d:d + 1],
                          lambda d: lnp2b_t[:, d:d + 1])

        # =================== transformer layers ===================
        for l in range(DEPTH):
            h = big.tile([P, DTILES, TOK], DT_H, name="bigbuf")
            with nc.named_scope(f"L{l}_lna"):
                layer_norm_fm(lambda d: x[:, d, :], lambda d: h[:, d, :],
                              DTILES, DIM, TOK,
                              lambda d: lnag_t[:, l, d:d + 1],
                              lambda d: lnab_t[:, l, d:d + 1])

            et_sb = one1.tile([P, SEQT, SEQ], DT_F, name="et_sb")
            nc.sync.dma_start(out=et_sb,
                              in_=par["etab"][l].rearrange("jt p i -> p jt i"))

            of = big.tile([P, DTILES, TOK], DT_H, name="bigbuf")
            wq3 = par["wqkv"][l].rearrange("(ko p) m -> p ko m", p=P)
            for b in range(BPC):
                tsl = bass.ts(b, 512)
                with tc.tile_pool(name=f"qkv{l}_{b}", bufs=1) as qkvp:
                    qf = qkvp.tile([P, DTILES, 512], DT_H, name="qf")
                    kf = qkvp.tile([P, DTILES, 512], DT_H, name="kf")
                    vt = qkvp.tile([P, SEQT, INNER], DT_H, name="vt")
                    # q, k feature-major (k scaled by 1/sqrt(dhead))
                    for c in range(2 * DTILES):
                        wt = wsm.tile([P, KTILES, P], DT_H, name="wsm_t")
                        nc.sync.dma_start(out=wt, in_=wq3[:, :, bass.ts(c, P)])
                        pq = psm.tile([P, 512], DT_F, name="pmain")
                        for kt in range(KTILES):
                            nc.tensor.matmul(pq, wt[:, kt, :], h[:, kt, tsl],
                                             start=(kt == 0), stop=(kt == 7))
                        if c < DTILES:
                            nc.vector.tensor_copy(qf[:, c, :], pq)
                        else:
                            nc.vector.tensor_scalar(
                                kf[:, c - DTILES, :], pq,
                                float(DHEAD) ** -0.5, None, OP.mult)
                    # v token-major with ones column per head
                    for nh in range(2):
                        pvs = [psm.tile([P, 512], DT_F, name="pmain")
                               for _ in range(SEQT)]
                        for kt in range(KTILES):
                            wv = wsm.tile([P, 512], DT_H, name="wsm_t")
                            nc.sync.dma_start(
                                out=wv,
                                in_=wq3[:, kt, 2 * INNER + nh * 512:
                                        2 * INNER + (nh + 1) * 512])
                            for tt in range(SEQT):
                                nc.tensor.matmul(
                                    pvs[tt],
                                    h[:, kt, b * 512 + tt * P:
                                      b * 512 + (tt + 1) * P],
                                    wv,
                                    start=(kt == 0), stop=(kt == 7))
                        for tt in range(SEQT):
                            nc.vector.tensor_copy(
                                vt[:, tt, nh * 512:(nh + 1) * 512], pvs[tt])

                    for hd in range(HEADS):
                        po = (hd % 2) * 64
                        dt = hd // 2
                        sc = [psm.tile([P, 512], DT_F, name="pmain")
                              for _ in range(SEQT)]
                        # causal: j-tile jt only attends to i >= jt*P
                        for jt in range(SEQT):
                            i0 = jt * P
                            nc.tensor.matmul(
                                sc[jt][:, i0:],
                                kf[po:po + 64, dt, bass.ts(jt, P)],
                                qf[po:po + 64, dt, i0:],
                                start=True, stop=True)
                        etr = sm2.tile([P, SEQT, 512], DT_H, name="etr")
                        for jt in range(SEQT):
                            i0 = jt * P
                            ex = sm2.tile([P, 512], DT_F, name="ex")
                            nc.scalar.activation(ex[:, i0:], sc[jt][:, i0:],
                                                 FX.Exp)
                            nc.vector.tensor_mul(
                                etr[:, jt, i0:], ex[:, i0:],
                                et_sb[:, jt, i0:])
                        ot = pso.tile([P, 512], DT_F, name="ot")
                        den = pst.tile([1, 512], DT_F, name="ps0")
                        for jt in range(SEQT):
                            i0 = jt * P
                            nc.tensor.matmul(
                                ot[0:64, i0:],
                                vt[:, jt, hd * 64:(hd + 1) * 64],
                                etr[:, jt, i0:],
                                start=(jt == 0), stop=(jt == SEQT - 1))
                            nc.tensor.matmul(
                                den[:, i0:], ones_h, etr[:, jt, i0:],
                                start=(jt == 0), stop=(jt == SEQT - 1))
                        adr1 = stats.tile([1, 512], DT_F, name="adr1")
                        nc.vector.reciprocal_approx_fast(out=adr1, in_=den)
                        adrb = stats.tile([64, 512], DT_F, name="adrb")
                        nc.gpsimd.partition_broadcast(adrb, adr1)
                        if po == 0:
                            nc.vector.tensor_mul(
                                of[0:64, dt, tsl], ot[0:64, :], adrb)
                        else:
                            # DVE can't shift partitions: normalize at base 0,
                            # DMA-copy up to partitions 64:128.
                            otmp = sm2.tile([64, 512], DT_H, name="otmp")
                            nc.vector.tensor_mul(otmp, ot[0:64, :], adrb)
                            nc.sync.dma_start(
                                out=of[64:128, dt, tsl], in_=otmp)

            wo3 = par["wo"][l].rearrange("(ko p) m -> p ko m", p=P)
            for dt in range(DTILES):
                wt = wsm.tile([P, KTILES, P], DT_H, name="wsm_t")
                nc.sync.dma_start(out=wt, in_=wo3[:, :, bass.ts(dt, P)])
                for b in range(BPC):
                    pq = psm.tile([P, 512], DT_F, name="pmain")
                    for kt in range(KTILES):
                        nc.tensor.matmul(
                            pq, wt[:, kt, :], of[:, kt, bass.ts(b, 512)],
                            start=(kt == 0), stop=(kt == 7))
                    nc.scalar.activation(pq, pq, FX.Identity,
                                         bias=bov_t[:, l, dt:dt + 1])
                    nc.vector.tensor_add(
                        x[:, dt, bass.ts(b, 512)], pq,
                        x[:, dt, bass.ts(b, 512)])

            h2 = big.tile([P, DTILES, TOK], DT_H, name="bigbuf")
            layer_norm_fm(lambda d: x[:, d, :], lambda d: h2[:, d, :],
                          DTILES, DIM, TOK,
                          lambda d: lnfg_t[:, l, d:d + 1],
                          lambda d: lnfb_t[:, l, d:d + 1])
            w13 = par["w1"][l].rearrange("(ko p) m -> p ko m", p=P)
            w23 = par["w2"][l].rearrange("(ko p) m -> p ko m", p=P)
            for th in range(2):
                tsl = bass.ts(th, 512)
                with tc.tile_pool(name=f"h1p{l}_{th}", bufs=1) as h1p:
                    h1r = h1p.tile([P, MTILES, 512], DT_H, name="h1r")
                    for mt in range(MTILES):
                        wt = wsm.tile([P, KTILES, P], DT_H, name="wsm_t")
                        nc.sync.dma_start(out=wt,
                                          in_=w13[:, :, bass.ts(mt, P)])
                        pq = psm.tile([P, 512], DT_F, name="pmain")
                        for kt in range(KTILES):
                            nc.tensor.matmul(pq, wt[:, kt, :], h2[:, kt, tsl],
                                             start=(kt == 0), stop=(kt == 7))
                        nc.scalar.activation(
                            h1r[:, mt, :], pq, FX.Gelu,
                            bias=b1v_t[:, l, mt:mt + 1], scale=1.0)
                    for dt in range(DTILES):
                        pq = psm.tile([P, 512], DT_F, name="pmain")
                        for kh in range(2):
                            wt = wsm.tile([P, 16, P], DT_H, name="wsm_t")
                            nc.sync.dma_start(
                                out=wt,
                                in_=w23[:, kh * 16:(kh + 1) * 16,
                                        bass.ts(dt, P)])
                            for k2 in range(16):
                                kt = kh * 16 + k2
                                nc.tensor.matmul(
                                    pq, wt[:, k2, :], h1r[:, kt, :],
                                    start=(kt == 0), stop=(kt == 31))
                        nc.scalar.activation(pq, pq, FX.Identity,
                                             bias=b2v_t[:, l, dt:dt + 1])
                        nc.vector.tensor_add(
                            x[:, dt, tsl], pq, x[:, dt, tsl])

        # =================== head ===================
        ho = big.tile([P, DTILES, TOK], DT_R, name="bigbuf")
        layer_norm_fm(lambda d: x[:, d, :], lambda d: ho[:, d, :],
                      DTILES, DIM, TOK,
                      lambda d: lnog_t[:, d:d + 1],
                      lambda d: lnob_t[:, d:d + 1])
        wp3 = par["wproj"].rearrange("(ko p) m -> p ko m", p=P)
        wt = wsm.tile([P, KTILES, NCLS], DT_R, name="wsm_t")
        nc.sync.dma_start(out=wt, in_=wp3)
        out_sb = one1.tile([NCLS, TOK], DT_F, name="out_sb")
        for th in range(2):
            pq = pso.tile([P, 512], DT_F, name="ot")
            for kt in range(KTILES):
                nc.tensor.matmul(pq[0:NCLS, :], wt[:, kt, :],
                                 ho[:, kt, bass.ts(th, 512)],
                                 start=(kt == 0), stop=(kt == 7))
            nc.scalar.activation(out_sb[:, bass.ts(th, 512)], pq[0:NCLS, :],
                                 FX.Identity, bias=bproj_t)
        nc.sync.dma_start(out=par["out"][:, :], in_=out_sb)


# ============================================================
# host side
# ============================================================

_NC_CACHE = None


def _host_band():
    tt = np.arange(KSIZE, dtype=np.float64)
    kern = np.exp(-0.5 * ((tt - (KSIZE - 1) / 2.0) / SIGMA) ** 2)
    kern = (kern / kern.sum()).astype(np.float32)
    pad_l = (KSIZE - 1) // 2  # 9
    nt = T // P
    bandc = np.zeros((nt, 3, P, P), dtype=np.float32)
    for ct in range(nt):
        for s in range(3):
            kt = ct - 1 + s
            if not (0 <= kt < nt):
                continue
            rows = np.arange(kt * P, (kt + 1) * P)
            cols = np.arange(ct * P, (ct + 1) * P)
            d = rows[:, None] - cols[None, :] + pad_l
            m = (d >= 0) & (d < KSIZE)
            blk = np.zeros((P, P), np.float32)
            blk[m] = kern[d[m]]
            bandc[ct, s] = blk
    return bandc


def _host_etab(rel_tab):
    i = np.arange(SEQ)
    j = i[:, None]
    rel = np.clip(i[None, :] - j, -(MAXREL - 1), MAXREL - 1) + MAXREL - 1
    et = np.zeros((DEPTH, SEQ, SEQ), dtype=np.float32)
    for l in range(DEPTH):
        e = np.exp(rel_tab[l][rel])
        e[j > i[None, :]] = 0.0
        et[l] = e
    return et.reshape(DEPTH, SEQT, P, SEQ)


def kernel(**inputs):
    global _NC_CACHE
    if _NC_CACHE is None:
        _NC_CACHE = build_nc()
    nc = _NC_CACHE

    f32 = lambda a: np.ascontiguousarray(np.asarray(a, dtype=np.float32))
    import ml_dtypes
    shared = {
        "band": _host_band(),
        "etab": _host_etab(f32(inputs["rel_tab"])),
        "wpe": f32(inputs["W_pe"]),
        "wqkv": f32(inputs["Wqkv"]).astype(ml_dtypes.bfloat16),
        "wo": f32(inputs["Wo"]).astype(ml_dtypes.bfloat16),
        "w1": f32(inputs["W1"]).astype(ml_dtypes.bfloat16),
        "w2": f32(inputs["W2"]).astype(ml_dtypes.bfloat16),
        "wproj": f32(inputs["Wproj"]),
        "lnp1g": f32(inputs["ln_p1_g"]), "lnp1b": f32(inputs["ln_p1_b"]),
        "bpe": f32(inputs["b_pe"]),
        "lnp2g": f32(inputs["ln_p2_g"]), "lnp2b": f32(inputs["ln_p2_b"]),
        "lnag": f32(inputs["ln_a_g"]), "lnab": f32(inputs["ln_a_b"]),
        "lnfg": f32(inputs["ln_f_g"]), "lnfb": f32(inputs["ln_f_b"]),
        "bov": f32(inputs["bo"]), "b1v": f32(inputs["b1"]),
        "b2v": f32(inputs["b2"]),
        "lnog": f32(inputs["ln_o_g"]), "lnob": f32(inputs["ln_o_b"]),
        "bprojv": f32(inputs["bproj"]),
    }
    xfull = f32(inputs["neuralInput"])
    in_maps = []
    for c in range(NCORES):
        m = dict(shared)
        m["xin"] = np.ascontiguousarray(xfull[c * BPC:(c + 1) * BPC])
        in_maps.append(m)

    import os
    trace = bool(os.environ.get("BIT_TRACE"))
    res = run_bass_kernel_spmd(nc, in_maps, list(range(NCORES)), trace=trace)
    if trace:
        globals()["LAST_RESULT"] = res
    outs = []
    for c in range(NCORES):
        o = res.results[c]["out"]              # [NCLS, TOK]
        o = o.reshape(NCLS, BPC, SEQ).transpose(1, 2, 0)
        outs.append(o)
    return np.concatenate(outs, axis=0).astype(np.float32)


# revision 30
# speedup vs baseline: 1.3565x; 1.0100x over previous
"""Trainium2 Bass kernel for nn_BiT_Phoneme (dense transformer).

Data-parallel: 16 batch elems / 8 cores = 2 per core; each core runs the
full network on its 2 sequences (1024 "tokens"). Activations are kept
feature-major ([dim on partitions, tokens on free]) so matmuls chain
without transposes. The gaussian time-smoothing + patchify are a banded
matmul with a host-precomputed band matrix; patch extraction is a pure
access-pattern view. LayerNorm stats use ones-vector matmuls (partition
reduction); mean/rstd are broadcast back across partitions with K=1
matmuls into PSUM. Softmax runs transposed (keys on partitions) as
exp(scores)*E with E = exp(rel_bias)*causal_mask precomputed on host
(causal j-tiles only compute the valid i ranges); denominators come
from ones-vector matmuls over the probs; normalization is reciprocal +
gpsimd partition broadcast (odd heads are placed via an SBUF->SBUF DMA
partition shift, since DVE cannot cross partitions). Matmuls use
float32r (full PE rate at N>=256, ~1.5e-4 per-matmul accuracy; raw
fp32 bits are accepted for fp32r weight params). Attention probs / V /
h1 / W2 use bf16.
"""

import numpy as np

import concourse.bass as bass
import concourse.mybir as mybir
import concourse.tile as tile
from concourse import bacc
from concourse.bass_utils import run_bass_kernel_spmd

B, T, F = 16, 2048, 256
PH = 4
PATCH = 1024
DIM = 1024
DEPTH = 6
HEADS, DHEAD = 16, 64
INNER = 1024
MLP = 4096
NCLS = 41
MAXREL = 200
KSIZE, SIGMA = 20, 2.0
EPS = 1e-5
SEQ = T // PH              # 512
NCORES = 8
BPC = B // NCORES          # 2
TOK = BPC * SEQ            # 1024
P = 128

DT_R = mybir.dt.float32r
DT_F = mybir.dt.float32
DT_H = mybir.dt.bfloat16
FX = mybir.ActivationFunctionType
OP = mybir.AluOpType

DTILES = DIM // P          # 8
KTILES = DIM // P          # 8
MTILES = MLP // P          # 32
SEQT = SEQ // P            # 4


def build_nc():
    nc = bacc.Bacc(None, target_bir_lowering=False)

    par = {}
    def dp(name, shape, dtype, is_out=False):
        par[name] = nc.declare_dram_parameter(name, list(shape), dtype, isOutput=is_out)
        return par[name]

    dp("xin", (BPC, T, F), DT_R)
    dp("band", (T // P, 3, P, P), DT_R)
    dp("etab", (DEPTH, SEQT, P, SEQ), DT_F)
    dp("wpe", (PATCH, DIM), DT_R)
    dp("wqk_t", (DEPTH, 16, P, KTILES, P), DT_H)
    dp("wv_t", (DEPTH, 2, KTILES, P, 512), DT_H)
    dp("wot", (DEPTH, DTILES, P, KTILES, P), DT_H)
    dp("w1t", (DEPTH, MTILES, P, KTILES, P), DT_H)
    dp("w2t", (DEPTH, DTILES, 2, P, 16, P), DT_H)
    dp("wproj", (DIM, NCLS), DT_R)
    for nm, shp in [("lnp1g", (PATCH,)), ("lnp1b", (PATCH,)), ("bpe", (DIM,)),
                    ("lnp2g", (DIM,)), ("lnp2b", (DIM,)),
                    ("lnag", (DEPTH, DIM)), ("lnab", (DEPTH, DIM)),
                    ("lnfg", (DEPTH, DIM)), ("lnfb", (DEPTH, DIM)),
                    ("bov", (DEPTH, DIM)), ("b1v", (DEPTH, MLP)),
                    ("b2v", (DEPTH, DIM)), ("lnog", (DIM,)), ("lnob", (DIM,)),
                    ("bprojv", (NCLS,))]:
        dp(nm, shp, DT_F)
    dp("out", (NCLS, TOK), DT_F, is_out=True)

    with tile.TileContext(nc) as tc:
        _emit(nc, tc, par)
    nc.compile()
    return nc


def _emit(nc, tc, par):
    import contextlib
    ctx = contextlib.ExitStack()
    with ctx:
        const = ctx.enter_context(tc.tile_pool(name="const", bufs=1))
        xpool = ctx.enter_context(tc.tile_pool(name="xpool", bufs=1))
        big = ctx.enter_context(tc.tile_pool(name="big", bufs=2))
        wsm = ctx.enter_context(tc.tile_pool(name="wsm", bufs=4))
        stats = ctx.enter_context(tc.tile_pool(name="stats", bufs=1))
        sm2 = ctx.enter_context(tc.tile_pool(name="sm2", bufs=2))
        one1 = ctx.enter_context(tc.tile_pool(name="one1", bufs=1))
        atp = ctx.enter_context(tc.tile_pool(name="atp", bufs=3))
        psm = ctx.enter_context(tc.tile_pool(name="psm", bufs=4, space="PSUM"))
        pso = ctx.enter_context(tc.tile_pool(name="pso", bufs=2, space="PSUM"))
        pst = ctx.enter_context(tc.tile_pool(name="pst", bufs=1, space="PSUM"))

        ones_r = const.tile([P, 1], DT_R, name="ones_r")
        nc.vector.memset(ones_r.bitcast(mybir.dt.uint32), 0x3F800000)
        ones_h = const.tile([P, 1], DT_H, name="ones_h")
        nc.vector.memset(ones_h.bitcast(mybir.dt.uint16), 0x3F80)
        ones_row = const.tile([1, P], DT_R, name="ones_row")
        nc.vector.memset(ones_row.bitcast(mybir.dt.uint32), 0x3F800000)
        epst = const.tile([1, 1], DT_F, name="epst")
        nc.vector.memset(epst, EPS)

        def load_vec(nm, width):
            d = par[nm]
            if len(d.shape) == 1:
                tl = const.tile([P, width // P], DT_F, name=nm + "_t")
                nc.sync.dma_start(out=tl, in_=d.rearrange("(o p) -> p o", p=P))
            else:
                L = d.shape[0]
                tl = const.tile([P, L, width // P], DT_F, name=nm + "_t")
                nc.sync.dma_start(out=tl, in_=d.rearrange("l (o p) -> p l o", p=P))
            return tl

        lnp1g_t = load_vec("lnp1g", PATCH)
        lnp1b_t = load_vec("lnp1b", PATCH)
        bpe_t = load_vec("bpe", DIM)
        lnp2g_t = load_vec("lnp2g", DIM)
        lnp2b_t = load_vec("lnp2b", DIM)
        lnag_t = load_vec("lnag", DIM)
        lnab_t = load_vec("lnab", DIM)
        lnfg_t = load_vec("lnfg", DIM)
        lnfb_t = load_vec("lnfb", DIM)
        bov_t = load_vec("bov", DIM)
        b1v_t = load_vec("b1v", MLP)
        b2v_t = load_vec("b2v", DIM)
        lnog_t = load_vec("lnog", DIM)
        lnob_t = load_vec("lnob", DIM)
        bproj_t = const.tile([NCLS, 1], DT_F, name="bproj_t")
        nc.sync.dma_start(out=bproj_t,
                          in_=par["bprojv"].rearrange("(p o) -> p o", o=1))

        x = xpool.tile([P, DTILES, TOK], DT_R, name="x")

        # ---- layernorm over partitions+tiles (feature-major) ----
        # views(d) -> [P, width] fp32r source slices (ntiles of them);
        # writes dst_fn(d) slices, width tokens, nh 512-halves.
        def layer_norm_fm(views, dst_fn, ntiles, D, width, g_fn, b_fn):
            nh = width // 512
            st = stats.tile([1, 2, TOK], DT_R, name="st")
            mu, rstd = st[:, 0, :width], st[:, 1, :width]
            vt32 = stats.tile([1, TOK], DT_F, name="vt32")
            vtmp = vt32[:, :width]
            for th in range(nh):
                sl = bass.ts(th, 512)
                ps0 = pst.tile([1, 512], DT_F, name="ps0")
                ps1 = pst.tile([1, 512], DT_F, name="ps1")
                for d in range(ntiles):
                    v = views(d)[:, sl]
                    sq = sm2.tile([P, 512], DT_R, name="sq")
                    nc.vector.tensor_mul(sq, v, v)
                    nc.tensor.matmul(ps0, ones_r, v,
                                     start=(d == 0), stop=(d == ntiles - 1))
                    nc.tensor.matmul(ps1, ones_r, sq,
                                     start=(d == 0), stop=(d == ntiles - 1))
                nc.vector.tensor_scalar(mu[:, sl], ps0, 1.0 / D, None, OP.mult)
                # rstd slot <- mu^2 (scratch), vt32 <- E[x^2] - mu^2
                nc.vector.tensor_scalar(vtmp[:, sl], ps1, 1.0 / D, None,
                                        OP.mult)
                nc.vector.tensor_mul(rstd[:, sl], mu[:, sl], mu[:, sl])
                nc.vector.tensor_sub(vtmp[:, sl], vtmp[:, sl], rstd[:, sl])
            nc.scalar.activation(vtmp, vtmp, FX.Sqrt, bias=epst, scale=1.0)
            with nc.allow_low_precision(reason="fp32r rstd feeds matmul bcast"):
                nc.vector.reciprocal(rstd, vtmp)
            for th in range(nh):
                sl = bass.ts(th, 512)
                # broadcast mu/rstd across partitions via K=1 matmul
                mps = psm.tile([P, 512], DT_F, name="pmain")
                rps = psm.tile([P, 512], DT_F, name="pmain")
                nc.tensor.matmul(mps, ones_row, mu[:, sl],
                                 start=True, stop=True)
                nc.tensor.matmul(rps, ones_row, rstd[:, sl],
                                 start=True, stop=True)
                for d in range(ntiles):
                    tmp = sm2.tile([P, 512], DT_F, name="lntmp")
                    nc.vector.tensor_sub(tmp, views(d)[:, sl], mps)
                    nc.vector.tensor_mul(tmp, tmp, rps)
                    nc.vector.tensor_scalar(
                        dst_fn(d)[:, sl], tmp, g_fn(d), b_fn(d),
                        OP.mult, OP.add)

        # =================== embedding ===================
        xin, band = par["xin"], par["band"]
        with (
            tc.tile_pool(name="sfp", bufs=1) as sfp,
            tc.tile_pool(name="pnp", bufs=1) as pnp,
        ):
            x0 = big.tile([P, DTILES, TOK], DT_R, name="bigbuf")
            for b in range(BPC):
                sf = sfp.tile([P, 2, T], DT_R, name="sf")
                for fh in range(2):
                    for g4 in range(T // 512):
                        pg = psm.tile([P, 512], DT_F, name="pmain")
                        for q in range(4):
                            ct = g4 * 4 + q
                            bt = wsm.tile([P, 3, P], DT_R, name="wsm_t")
                            nc.sync.dma_start(
                                out=bt,
                                in_=band[ct].rearrange("s p q -> p s q"))
                            svals = [s for s in range(3)
                                     if 0 <= ct - 1 + s < T // P]
                            for si, s in enumerate(svals):
                                kt = ct - 1 + s
                                xa = wsm.tile([P, P], DT_R, name="wsm_t")
                                nc.sync.dma_start(
                                    out=xa,
                                    in_=xin[b, bass.ts(kt, P),
                                            bass.ts(fh, P)])
                                nc.tensor.matmul(
                                    pg[:, bass.ts(q, P)],
                                    xa,
                                    bt[:, s, :],
                                    start=(si == 0),
                                    stop=(si == len(svals) - 1))
                        nc.vector.tensor_copy(sf[:, fh, bass.ts(g4, 512)], pg)

                def pview(pt):
                    i, fh = pt // 2, pt % 2
                    return sf[:, fh, :].rearrange(
                        "p (s four) -> p four s", four=PH)[:, i, :]

                pn = pnp.tile([P, 8, 512], DT_R, name="pn")
                layer_norm_fm(pview, lambda d: pn[:, d, :], 8, PATCH, 512,
                              lambda d: lnp1g_t[:, d:d + 1],
                              lambda d: lnp1b_t[:, d:d + 1])

                for dt in range(DTILES):
                    wt = wsm.tile([P, KTILES, P], DT_R, name="wsm_t")
                    nc.sync.dma_start(
                        out=wt,
                        in_=par["wpe"].rearrange("(ko p) m -> p ko m", p=P)[
                            :, :, bass.ts(dt, P)])
                    pq = psm.tile([P, 512], DT_F, name="pmain")
                    for kt in range(KTILES):
                        nc.tensor.matmul(pq, wt[:, kt, :], pn[:, kt, :],
                                         start=(kt == 0), stop=(kt == 7))
                    nc.vector.tensor_scalar(
                        x0[:, dt, bass.ts(b, 512)], pq,
                        bpe_t[:, dt:dt + 1], None, OP.add)

            layer_norm_fm(lambda d: x0[:, d, :], lambda d: x[:, d, :],
                          DTILES, DIM, TOK,
                          lambda d: lnp2g_t[:, d:d + 1],
                          lambda d: lnp2b_t[:, d:d + 1])

        # =================== transformer layers ===================
        for l in range(DEPTH):
            h = big.tile([P, DTILES, TOK], DT_H, name="bigbuf")
            with nc.named_scope(f"L{l}_lna"):
                layer_norm_fm(lambda d: x[:, d, :], lambda d: h[:, d, :],
                              DTILES, DIM, TOK,
                              lambda d: lnag_t[:, l, d:d + 1],
                              lambda d: lnab_t[:, l, d:d + 1])

            et_sb = one1.tile([P, SEQT, SEQ], DT_F, name="et_sb")
            nc.sync.dma_start(out=et_sb,
                              in_=par["etab"][l].rearrange("jt p i -> p jt i"))

            of = big.tile([P, DTILES, TOK], DT_H, name="bigbuf")
            for b in range(BPC):
                tsl = bass.ts(b, 512)
                with tc.tile_pool(name=f"qkv{l}_{b}", bufs=1) as qkvp:
                    qf = qkvp.tile([P, DTILES, 512], DT_H, name="qf")
                    kf = qkvp.tile([P, DTILES, 512], DT_H, name="kf")
                    vt = qkvp.tile([P, SEQT, INNER], DT_H, name="vt")
                    # q, k feature-major (k scaled by 1/sqrt(dhead))
                    for c in range(2 * DTILES):
                        wt = wsm.tile([P, KTILES, P], DT_H, name="wsm_t")
                        nc.sync.dma_start(out=wt, in_=par["wqk_t"][l, c])
                        pq = psm.tile([P, 512], DT_F, name="pmain")
                        for kt in range(KTILES):
                            nc.tensor.matmul(pq, wt[:, kt, :], h[:, kt, tsl],
                                             start=(kt == 0), stop=(kt == 7))
                        if c < DTILES:
                            nc.vector.tensor_copy(qf[:, c, :], pq)
                        else:
                            nc.vector.tensor_scalar(
                                kf[:, c - DTILES, :], pq,
                                float(DHEAD) ** -0.5, None, OP.mult)
                    # v token-major with ones column per head
                    for nh in range(2):
                        pvs = [psm.tile([P, 512], DT_F, name="pmain")
                               for _ in range(SEQT)]
                        for kt in range(KTILES):
                            wv = wsm.tile([P, 512], DT_H, name="wsm_t")
                            nc.sync.dma_start(
                                out=wv, in_=par["wv_t"][l, nh, kt])
                            for tt in range(SEQT):
                                nc.tensor.matmul(
                                    pvs[tt],
                                    h[:, kt, b * 512 + tt * P:
                                      b * 512 + (tt + 1) * P],
                                    wv,
                                    start=(kt == 0), stop=(kt == 7))
                        for tt in range(SEQT):
                            nc.vector.tensor_copy(
                                vt[:, tt, nh * 512:(nh + 1) * 512], pvs[tt])

                    for hd in range(HEADS):
                        po = (hd % 2) * 64
                        dt = hd // 2
                        sc = [psm.tile([P, 512], DT_F, name="pmain")
                              for _ in range(SEQT)]
                        # causal: j-tile jt only attends to i >= jt*P
                        for jt in range(SEQT):
                            i0 = jt * P
                            nc.tensor.matmul(
                                sc[jt][:, i0:],
                                kf[po:po + 64, dt, bass.ts(jt, P)],
                                qf[po:po + 64, dt, i0:],
                                start=True, stop=True)
                        etr = atp.tile([P, SEQT, 512], DT_H, name="etr")
                        for jt in range(SEQT):
                            i0 = jt * P
                            ex = atp.tile([P, 512], DT_F, name="ex")
                            nc.scalar.activation(ex[:, i0:], sc[jt][:, i0:],
                                                 FX.Exp)
                            nc.vector.tensor_mul(
                                etr[:, jt, i0:], ex[:, i0:],
                                et_sb[:, jt, i0:])
                        ot = pso.tile([P, 512], DT_F, name="ot")
                        den = pst.tile([1, 512], DT_F, name="ps0")
                        for jt in range(SEQT):
                            i0 = jt * P
                            nc.tensor.matmul(
                                ot[0:64, i0:],
                                vt[:, jt, hd * 64:(hd + 1) * 64],
                                etr[:, jt, i0:],
                                start=(jt == 0), stop=(jt == SEQT - 1))
                            nc.tensor.matmul(
                                den[:, i0:], ones_h, etr[:, jt, i0:],
                                start=(jt == 0), stop=(jt == SEQT - 1))
                        adr1 = stats.tile([1, 512], DT_F, name="adr1")
                        nc.vector.reciprocal_approx_fast(out=adr1, in_=den)
                        adrb = stats.tile([64, 512], DT_F, name="adrb")
                        nc.gpsimd.partition_broadcast(adrb, adr1)
                        if po == 0:
                            nc.vector.tensor_mul(
                                of[0:64, dt, tsl], ot[0:64, :], adrb)
                        else:
                            # DVE can't shift partitions: normalize at base 0,
                            # DMA-copy up to partitions 64:128.
                            otmp = sm2.tile([64, 512], DT_H, name="otmp")
                            nc.vector.tensor_mul(otmp, ot[0:64, :], adrb)
                            nc.sync.dma_start(
                                out=of[64:128, dt, tsl], in_=otmp)

            for dt in range(DTILES):
                wt = wsm.tile([P, KTILES, P], DT_H, name="wsm_t")
                nc.sync.dma_start(out=wt, in_=par["wot"][l, dt])
                for b in range(BPC):
                    pq = psm.tile([P, 512], DT_F, name="pmain")
                    for kt in range(KTILES):
                        nc.tensor.matmul(
                            pq, wt[:, kt, :], of[:, kt, bass.ts(b, 512)],
                            start=(kt == 0), stop=(kt == 7))
                    nc.scalar.activation(pq, pq, FX.Identity,
                                         bias=bov_t[:, l, dt:dt + 1])
                    nc.vector.tensor_add(
                        x[:, dt, bass.ts(b, 512)], pq,
                        x[:, dt, bass.ts(b, 512)])

            h2 = big.tile([P, DTILES, TOK], DT_H, name="bigbuf")
            layer_norm_fm(lambda d: x[:, d, :], lambda d: h2[:, d, :],
                          DTILES, DIM, TOK,
                          lambda d: lnfg_t[:, l, d:d + 1],
                          lambda d: lnfb_t[:, l, d:d + 1])

            for th in range(2):
                tsl = bass.ts(th, 512)
                with tc.tile_pool(name=f"h1p{l}_{th}", bufs=1) as h1p:
                    h1r = h1p.tile([P, MTILES, 512], DT_H, name="h1r")
                    for mt in range(MTILES):
                        wt = wsm.tile([P, KTILES, P], DT_H, name="wsm_t")
                        nc.sync.dma_start(out=wt, in_=par["w1t"][l, mt])
                        pq = psm.tile([P, 512], DT_F, name="pmain")
                        for kt in range(KTILES):
                            nc.tensor.matmul(pq, wt[:, kt, :], h2[:, kt, tsl],
                                             start=(kt == 0), stop=(kt == 7))
                        nc.scalar.activation(
                            h1r[:, mt, :], pq, FX.Gelu,
                            bias=b1v_t[:, l, mt:mt + 1], scale=1.0)
                    for dt in range(DTILES):
                        pq = psm.tile([P, 512], DT_F, name="pmain")
                        for kh in range(2):
                            wt = wsm.tile([P, 16, P], DT_H, name="wsm_t")
                            nc.sync.dma_start(
                                out=wt, in_=par["w2t"][l, dt, kh])
                            for k2 in range(16):
                                kt = kh * 16 + k2
                                nc.tensor.matmul(
                                    pq, wt[:, k2, :], h1r[:, kt, :],
                                    start=(kt == 0), stop=(kt == 31))
                        nc.scalar.activation(pq, pq, FX.Identity,
                                             bias=b2v_t[:, l, dt:dt + 1])
                        nc.vector.tensor_add(
                            x[:, dt, tsl], pq, x[:, dt, tsl])

        # =================== head ===================
        ho = big.tile([P, DTILES, TOK], DT_R, name="bigbuf")
        layer_norm_fm(lambda d: x[:, d, :], lambda d: ho[:, d, :],
                      DTILES, DIM, TOK,
                      lambda d: lnog_t[:, d:d + 1],
                      lambda d: lnob_t[:, d:d + 1])
        wp3 = par["wproj"].rearrange("(ko p) m -> p ko m", p=P)
        wt = wsm.tile([P, KTILES, NCLS], DT_R, name="wsm_t")
        nc.sync.dma_start(out=wt, in_=wp3)
        out_sb = one1.tile([NCLS, TOK], DT_F, name="out_sb")
        for th in range(2):
            pq = pso.tile([P, 512], DT_F, name="ot")
            for kt in range(KTILES):
                nc.tensor.matmul(pq[0:NCLS, :], wt[:, kt, :],
                                 ho[:, kt, bass.ts(th, 512)],
                                 start=(kt == 0), stop=(kt == 7))
            nc.scalar.activation(out_sb[:, bass.ts(th, 512)], pq[0:NCLS, :],
                                 FX.Identity, bias=bproj_t)
        nc.sync.dma_start(out=par["out"][:, :], in_=out_sb)


# ============================================================
# host side
# ============================================================

_NC_CACHE = None


def _bf16(a):
    import ml_dtypes
    return np.ascontiguousarray(a.astype(ml_dtypes.bfloat16))


def _pack_qk(w):      # [D, DIM, 3072] -> [D, 16, P, 8, P]
    v = w[:, :, :2048].reshape(DEPTH, 8, P, 16, P).transpose(0, 3, 2, 1, 4)
    return _bf16(v)


def _pack_v(w):       # -> [D, 2, 8, P, 512]
    v = w[:, :, 2048:].reshape(DEPTH, 8, P, 2, 512).transpose(0, 3, 1, 2, 4)
    return _bf16(v)


def _pack_kxm(w):     # [D, K, M] -> [D, M//P, P, K//P, P]
    D, K, M = w.shape
    v = w.reshape(D, K // P, P, M // P, P).transpose(0, 3, 2, 1, 4)
    return _bf16(v)


def _pack_w2(w):      # [D, 4096, 1024] -> [D, 8, 2, P, 16, P]
    v = w.reshape(DEPTH, 2, 16, P, 8, P).transpose(0, 4, 1, 3, 2, 5)
    return _bf16(v)


def _host_band():
    tt = np.arange(KSIZE, dtype=np.float64)
    kern = np.exp(-0.5 * ((tt - (KSIZE - 1) / 2.0) / SIGMA) ** 2)
    kern = (kern / kern.sum()).astype(np.float32)
    pad_l = (KSIZE - 1) // 2  # 9
    nt = T // P
    bandc = np.zeros((nt, 3, P, P), dtype=np.float32)
    for ct in range(nt):
        for s in range(3):
            kt = ct - 1 + s
            if not (0 <= kt < nt):
                continue
            rows = np.arange(kt * P, (kt + 1) * P)
            cols = np.arange(ct * P, (ct + 1) * P)
            d = rows[:, None] - cols[None, :] + pad_l
            m = (d >= 0) & (d < KSIZE)
            blk = np.zeros((P, P), np.float32)
            blk[m] = kern[d[m]]
            bandc[ct, s] = blk
    return bandc


def _host_etab(rel_tab):
    i = np.arange(SEQ)
    j = i[:, None]
    rel = np.clip(i[None, :] - j, -(MAXREL - 1), MAXREL - 1) + MAXREL - 1
    et = np.zeros((DEPTH, SEQ, SEQ), dtype=np.float32)
    for l in range(DEPTH):
        e = np.exp(rel_tab[l][rel])
        e[j > i[None, :]] = 0.0
        et[l] = e
    return et.reshape(DEPTH, SEQT, P, SEQ)


def kernel(**inputs):
    global _NC_CACHE
    if _NC_CACHE is None:
        _NC_CACHE = build_nc()
    nc = _NC_CACHE

    f32 = lambda a: np.ascontiguousarray(np.asarray(a, dtype=np.float32))
    import ml_dtypes
    shared = {
        "band": _host_band(),
        "etab": _host_etab(f32(inputs["rel_tab"])),
        "wpe": f32(inputs["W_pe"]),
        "wqk_t": _pack_qk(f32(inputs["Wqkv"])),
        "wv_t": _pack_v(f32(inputs["Wqkv"])),
        "wot": _pack_kxm(f32(inputs["Wo"])),
        "w1t": _pack_kxm(f32(inputs["W1"])),
        "w2t": _pack_w2(f32(inputs["W2"])),
        "wproj": f32(inputs["Wproj"]),
        "lnp1g": f32(inputs["ln_p1_g"]), "lnp1b": f32(inputs["ln_p1_b"]),
        "bpe": f32(inputs["b_pe"]),
        "lnp2g": f32(inputs["ln_p2_g"]), "lnp2b": f32(inputs["ln_p2_b"]),
        "lnag": f32(inputs["ln_a_g"]), "lnab": f32(inputs["ln_a_b"]),
        "lnfg": f32(inputs["ln_f_g"]), "lnfb": f32(inputs["ln_f_b"]),
        "bov": f32(inputs["bo"]), "b1v": f32(inputs["b1"]),
        "b2v": f32(inputs["b2"]),
        "lnog": f32(inputs["ln_o_g"]), "lnob": f32(inputs["ln_o_b"]),
        "bprojv": f32(inputs["bproj"]),
    }
    xfull = f32(inputs["neuralInput"])
    in_maps = []
    for c in range(NCORES):
        m = dict(shared)
        m["xin"] = np.ascontiguousarray(xfull[c * BPC:(c + 1) * BPC])
        in_maps.append(m)

    import os
    trace = bool(os.environ.get("BIT_TRACE"))
    res = run_bass_kernel_spmd(nc, in_maps, list(range(NCORES)), trace=trace)
    if trace:
        globals()["LAST_RESULT"] = res
    outs = []
    for c in range(NCORES):
        o = res.results[c]["out"]              # [NCLS, TOK]
        o = o.reshape(NCLS, BPC, SEQ).transpose(1, 2, 0)
        outs.append(o)
    return np.concatenate(outs, axis=0).astype(np.float32)


# revision 31
# speedup vs baseline: 1.3677x; 1.0082x over previous
"""Trainium2 Bass kernel for nn_BiT_Phoneme (dense transformer).

Data-parallel: 16 batch elems / 8 cores = 2 per core; each core runs the
full network on its 2 sequences (1024 "tokens"). Activations are kept
feature-major ([dim on partitions, tokens on free]) so matmuls chain
without transposes. The gaussian time-smoothing + patchify are a banded
matmul with a host-precomputed band matrix; patch extraction is a pure
access-pattern view. LayerNorm stats use ones-vector matmuls (partition
reduction); mean/rstd are broadcast back across partitions with K=1
matmuls into PSUM. Softmax runs transposed (keys on partitions) as
exp(scores)*E with E = exp(rel_bias)*causal_mask precomputed on host
(causal j-tiles only compute the valid i ranges); denominators come
from ones-vector matmuls over the probs; normalization is reciprocal +
gpsimd partition broadcast (odd heads are placed via an SBUF->SBUF DMA
partition shift, since DVE cannot cross partitions). Matmuls use
float32r (full PE rate at N>=256, ~1.5e-4 per-matmul accuracy; raw
fp32 bits are accepted for fp32r weight params). Attention probs / V /
h1 / W2 use bf16.
"""

import numpy as np

import concourse.bass as bass
import concourse.mybir as mybir
import concourse.tile as tile
from concourse import bacc
from concourse.bass_utils import run_bass_kernel_spmd

B, T, F = 16, 2048, 256
PH = 4
PATCH = 1024
DIM = 1024
DEPTH = 6
HEADS, DHEAD = 16, 64
INNER = 1024
MLP = 4096
NCLS = 41
MAXREL = 200
KSIZE, SIGMA = 20, 2.0
EPS = 1e-5
SEQ = T // PH              # 512
NCORES = 8
BPC = B // NCORES          # 2
TOK = BPC * SEQ            # 1024
P = 128

DT_R = mybir.dt.float32r
DT_F = mybir.dt.float32
DT_H = mybir.dt.bfloat16
FX = mybir.ActivationFunctionType
OP = mybir.AluOpType

DTILES = DIM // P          # 8
KTILES = DIM // P          # 8
MTILES = MLP // P          # 32
SEQT = SEQ // P            # 4


def build_nc():
    nc = bacc.Bacc(None, target_bir_lowering=False)

    par = {}
    def dp(name, shape, dtype, is_out=False):
        par[name] = nc.declare_dram_parameter(name, list(shape), dtype, isOutput=is_out)
        return par[name]

    dp("xin", (BPC, T, F), DT_R)
    dp("band", (T // P, 3, P, P), DT_R)
    dp("etab", (DEPTH, SEQT, P, SEQ), DT_F)
    dp("wpe", (PATCH, DIM), DT_R)
    dp("wqk_t", (DEPTH, 16, P, KTILES, P), DT_H)
    dp("wv_t", (DEPTH, 2, KTILES, P, 512), DT_H)
    dp("wot", (DEPTH, DTILES, P, KTILES, P), DT_H)
    dp("w1t", (DEPTH, MTILES, P, KTILES, P), DT_H)
    dp("w2t", (DEPTH, DTILES, 2, P, 16, P), DT_H)
    dp("wproj", (DIM, NCLS), DT_R)
    for nm, shp in [("lnp1g", (PATCH,)), ("lnp1b", (PATCH,)), ("bpe", (DIM,)),
                    ("lnp2g", (DIM,)), ("lnp2b", (DIM,)),
                    ("lnag", (DEPTH, DIM)), ("lnab", (DEPTH, DIM)),
                    ("lnfg", (DEPTH, DIM)), ("lnfb", (DEPTH, DIM)),
                    ("bov", (DEPTH, DIM)), ("b1v", (DEPTH, MLP)),
                    ("b2v", (DEPTH, DIM)), ("lnog", (DIM,)), ("lnob", (DIM,)),
                    ("bprojv", (NCLS,))]:
        dp(nm, shp, DT_F)
    dp("out", (NCLS, TOK), DT_F, is_out=True)

    with tile.TileContext(nc) as tc:
        _emit(nc, tc, par)
    nc.compile()
    return nc


def _emit(nc, tc, par):
    import contextlib
    ctx = contextlib.ExitStack()
    with ctx:
        const = ctx.enter_context(tc.tile_pool(name="const", bufs=1))
        xpool = ctx.enter_context(tc.tile_pool(name="xpool", bufs=1))
        big = ctx.enter_context(tc.tile_pool(name="big", bufs=2))
        wsm = ctx.enter_context(tc.tile_pool(name="wsm", bufs=4))
        stats = ctx.enter_context(tc.tile_pool(name="stats", bufs=1))
        sm2 = ctx.enter_context(tc.tile_pool(name="sm2", bufs=2))
        one1 = ctx.enter_context(tc.tile_pool(name="one1", bufs=1))
        atp = ctx.enter_context(tc.tile_pool(name="atp", bufs=3))
        psm = ctx.enter_context(tc.tile_pool(name="psm", bufs=3, space="PSUM"))
        pso = ctx.enter_context(tc.tile_pool(name="pso", bufs=3, space="PSUM"))
        pst = ctx.enter_context(tc.tile_pool(name="pst", bufs=1, space="PSUM"))

        ones_r = const.tile([P, 1], DT_R, name="ones_r")
        nc.vector.memset(ones_r.bitcast(mybir.dt.uint32), 0x3F800000)
        ones_h = const.tile([P, 1], DT_H, name="ones_h")
        nc.vector.memset(ones_h.bitcast(mybir.dt.uint16), 0x3F80)
        ones_row = const.tile([1, P], DT_R, name="ones_row")
        nc.vector.memset(ones_row.bitcast(mybir.dt.uint32), 0x3F800000)
        epst = const.tile([1, 1], DT_F, name="epst")
        nc.vector.memset(epst, EPS)

        def load_vec(nm, width):
            d = par[nm]
            if len(d.shape) == 1:
                tl = const.tile([P, width // P], DT_F, name=nm + "_t")
                nc.sync.dma_start(out=tl, in_=d.rearrange("(o p) -> p o", p=P))
            else:
                L = d.shape[0]
                tl = const.tile([P, L, width // P], DT_F, name=nm + "_t")
                nc.sync.dma_start(out=tl, in_=d.rearrange("l (o p) -> p l o", p=P))
            return tl

        lnp1g_t = load_vec("lnp1g", PATCH)
        lnp1b_t = load_vec("lnp1b", PATCH)
        bpe_t = load_vec("bpe", DIM)
        lnp2g_t = load_vec("lnp2g", DIM)
        lnp2b_t = load_vec("lnp2b", DIM)
        lnag_t = load_vec("lnag", DIM)
        lnab_t = load_vec("lnab", DIM)
        lnfg_t = load_vec("lnfg", DIM)
        lnfb_t = load_vec("lnfb", DIM)
        bov_t = load_vec("bov", DIM)
        b1v_t = load_vec("b1v", MLP)
        b2v_t = load_vec("b2v", DIM)
        lnog_t = load_vec("lnog", DIM)
        lnob_t = load_vec("lnob", DIM)
        bproj_t = const.tile([NCLS, 1], DT_F, name="bproj_t")
        nc.sync.dma_start(out=bproj_t,
                          in_=par["bprojv"].rearrange("(p o) -> p o", o=1))

        x = xpool.tile([P, DTILES, TOK], DT_R, name="x")

        # ---- layernorm over partitions+tiles (feature-major) ----
        # views(d) -> [P, width] fp32r source slices (ntiles of them);
        # writes dst_fn(d) slices, width tokens, nh 512-halves.
        def layer_norm_fm(views, dst_fn, ntiles, D, width, g_fn, b_fn):
            nh = width // 512
            st = stats.tile([1, 2, TOK], DT_R, name="st")
            mu, rstd = st[:, 0, :width], st[:, 1, :width]
            vt32 = stats.tile([1, TOK], DT_F, name="vt32")
            vtmp = vt32[:, :width]
            for th in range(nh):
                sl = bass.ts(th, 512)
                ps0 = pst.tile([1, 512], DT_F, name="ps0")
                ps1 = pst.tile([1, 512], DT_F, name="ps1")
                for d in range(ntiles):
                    v = views(d)[:, sl]
                    sq = sm2.tile([P, 512], DT_R, name="sq")
                    nc.vector.tensor_mul(sq, v, v)
                    nc.tensor.matmul(ps0, ones_r, v,
                                     start=(d == 0), stop=(d == ntiles - 1))
                    nc.tensor.matmul(ps1, ones_r, sq,
                                     start=(d == 0), stop=(d == ntiles - 1))
                nc.vector.tensor_scalar(mu[:, sl], ps0, 1.0 / D, None, OP.mult)
                # rstd slot <- mu^2 (scratch), vt32 <- E[x^2] - mu^2
                nc.vector.tensor_scalar(vtmp[:, sl], ps1, 1.0 / D, None,
                                        OP.mult)
                nc.vector.tensor_mul(rstd[:, sl], mu[:, sl], mu[:, sl])
                nc.vector.tensor_sub(vtmp[:, sl], vtmp[:, sl], rstd[:, sl])
            nc.scalar.activation(vtmp, vtmp, FX.Sqrt, bias=epst, scale=1.0)
            with nc.allow_low_precision(reason="fp32r rstd feeds matmul bcast"):
                nc.vector.reciprocal(rstd, vtmp)
            for th in range(nh):
                sl = bass.ts(th, 512)
                # broadcast mu/rstd across partitions via K=1 matmul
                mps = psm.tile([P, 512], DT_F, name="pmain")
                rps = psm.tile([P, 512], DT_F, name="pmain")
                nc.tensor.matmul(mps, ones_row, mu[:, sl],
                                 start=True, stop=True)
                nc.tensor.matmul(rps, ones_row, rstd[:, sl],
                                 start=True, stop=True)
                for d in range(ntiles):
                    tmp = sm2.tile([P, 512], DT_F, name="lntmp")
                    nc.vector.tensor_sub(tmp, views(d)[:, sl], mps)
                    nc.vector.tensor_mul(tmp, tmp, rps)
                    nc.vector.tensor_scalar(
                        dst_fn(d)[:, sl], tmp, g_fn(d), b_fn(d),
                        OP.mult, OP.add)

        # =================== embedding ===================
        xin, band = par["xin"], par["band"]
        with (
            tc.tile_pool(name="sfp", bufs=1) as sfp,
            tc.tile_pool(name="pnp", bufs=1) as pnp,
        ):
            x0 = big.tile([P, DTILES, TOK], DT_R, name="bigbuf")
            for b in range(BPC):
                sf = sfp.tile([P, 2, T], DT_R, name="sf")
                for fh in range(2):
                    for g4 in range(T // 512):
                        pg = psm.tile([P, 512], DT_F, name="pmain")
                        for q in range(4):
                            ct = g4 * 4 + q
                            bt = wsm.tile([P, 3, P], DT_R, name="wsm_t")
                            nc.sync.dma_start(
                                out=bt,
                                in_=band[ct].rearrange("s p q -> p s q"))
                            svals = [s for s in range(3)
                                     if 0 <= ct - 1 + s < T // P]
                            for si, s in enumerate(svals):
                                kt = ct - 1 + s
                                xa = wsm.tile([P, P], DT_R, name="wsm_t")
                                nc.sync.dma_start(
                                    out=xa,
                                    in_=xin[b, bass.ts(kt, P),
                                            bass.ts(fh, P)])
                                nc.tensor.matmul(
                                    pg[:, bass.ts(q, P)],
                                    xa,
                                    bt[:, s, :],
                                    start=(si == 0),
                                    stop=(si == len(svals) - 1))
                        nc.vector.tensor_copy(sf[:, fh, bass.ts(g4, 512)], pg)

                def pview(pt):
                    i, fh = pt // 2, pt % 2
                    return sf[:, fh, :].rearrange(
                        "p (s four) -> p four s", four=PH)[:, i, :]

                pn = pnp.tile([P, 8, 512], DT_R, name="pn")
                layer_norm_fm(pview, lambda d: pn[:, d, :], 8, PATCH, 512,
                              lambda d: lnp1g_t[:, d:d + 1],
                              lambda d: lnp1b_t[:, d:d + 1])

                for dt in range(DTILES):
                    wt = wsm.tile([P, KTILES, P], DT_R, name="wsm_t")
                    nc.sync.dma_start(
                        out=wt,
                        in_=par["wpe"].rearrange("(ko p) m -> p ko m", p=P)[
                            :, :, bass.ts(dt, P)])
                    pq = psm.tile([P, 512], DT_F, name="pmain")
                    for kt in range(KTILES):
                        nc.tensor.matmul(pq, wt[:, kt, :], pn[:, kt, :],
                                         start=(kt == 0), stop=(kt == 7))
                    nc.vector.tensor_scalar(
                        x0[:, dt, bass.ts(b, 512)], pq,
                        bpe_t[:, dt:dt + 1], None, OP.add)

            layer_norm_fm(lambda d: x0[:, d, :], lambda d: x[:, d, :],
                          DTILES, DIM, TOK,
                          lambda d: lnp2g_t[:, d:d + 1],
                          lambda d: lnp2b_t[:, d:d + 1])

        # =================== transformer layers ===================
        for l in range(DEPTH):
            h = big.tile([P, DTILES, TOK], DT_H, name="bigbuf")
            with nc.named_scope(f"L{l}_lna"):
                layer_norm_fm(lambda d: x[:, d, :], lambda d: h[:, d, :],
                              DTILES, DIM, TOK,
                              lambda d: lnag_t[:, l, d:d + 1],
                              lambda d: lnab_t[:, l, d:d + 1])

            et_sb = one1.tile([P, SEQT, SEQ], DT_F, name="et_sb")
            nc.sync.dma_start(out=et_sb,
                              in_=par["etab"][l].rearrange("jt p i -> p jt i"))

            of = big.tile([P, DTILES, TOK], DT_H, name="bigbuf")
            for b in range(BPC):
                tsl = bass.ts(b, 512)
                with tc.tile_pool(name=f"qkv{l}_{b}", bufs=1) as qkvp:
                    qf = qkvp.tile([P, DTILES, 512], DT_H, name="qf")
                    kf = qkvp.tile([P, DTILES, 512], DT_H, name="kf")
                    vt = qkvp.tile([P, SEQT, INNER], DT_H, name="vt")
                    # q, k feature-major (k scaled by 1/sqrt(dhead))
                    for c in range(2 * DTILES):
                        wt = wsm.tile([P, KTILES, P], DT_H, name="wsm_t")
                        nc.sync.dma_start(out=wt, in_=par["wqk_t"][l, c])
                        pq = psm.tile([P, 512], DT_F, name="pmain")
                        for kt in range(KTILES):
                            nc.tensor.matmul(pq, wt[:, kt, :], h[:, kt, tsl],
                                             start=(kt == 0), stop=(kt == 7))
                        if c < DTILES:
                            nc.vector.tensor_copy(qf[:, c, :], pq)
                        else:
                            nc.vector.tensor_scalar(
                                kf[:, c - DTILES, :], pq,
                                float(DHEAD) ** -0.5, None, OP.mult)
                    # v token-major with ones column per head
                    for nh in range(2):
                        for tth in range(2):
                            tts = (2 * tth, 2 * tth + 1)
                            pvs = [psm.tile([P, 512], DT_F, name="pmain")
                                   for _ in tts]
                            for kt in range(KTILES):
                                wv = wsm.tile([P, 512], DT_H, name="wsm_t")
                                nc.sync.dma_start(
                                    out=wv, in_=par["wv_t"][l, nh, kt])
                                for ti, tt in enumerate(tts):
                                    nc.tensor.matmul(
                                        pvs[ti],
                                        h[:, kt, b * 512 + tt * P:
                                          b * 512 + (tt + 1) * P],
                                        wv,
                                        start=(kt == 0), stop=(kt == 7))
                            for ti, tt in enumerate(tts):
                                nc.vector.tensor_copy(
                                    vt[:, tt, nh * 512:(nh + 1) * 512],
                                    pvs[ti])

                    for hd in range(HEADS):
                        po = (hd % 2) * 64
                        dt = hd // 2
                        sc = [psm.tile([P, 512], DT_F, name="pmain")
                              for _ in range(SEQT)]
                        # causal: j-tile jt only attends to i >= jt*P
                        for jt in range(SEQT):
                            i0 = jt * P
                            nc.tensor.matmul(
                                sc[jt][:, i0:],
                                kf[po:po + 64, dt, bass.ts(jt, P)],
                                qf[po:po + 64, dt, i0:],
                                start=True, stop=True)
                        etr = atp.tile([P, SEQT, 512], DT_H, name="etr")
                        for jt in range(SEQT):
                            i0 = jt * P
                            ex = atp.tile([P, 512], DT_F, name="ex")
                            nc.scalar.activation(ex[:, i0:], sc[jt][:, i0:],
                                                 FX.Exp)
                            nc.vector.tensor_mul(
                                etr[:, jt, i0:], ex[:, i0:],
                                et_sb[:, jt, i0:])
                        ot = pso.tile([P, 512], DT_F, name="ot")
                        den = pst.tile([1, 512], DT_F, name="ps0")
                        for jt in range(SEQT):
                            i0 = jt * P
                            nc.tensor.matmul(
                                ot[0:64, i0:],
                                vt[:, jt, hd * 64:(hd + 1) * 64],
                                etr[:, jt, i0:],
                                start=(jt == 0), stop=(jt == SEQT - 1))
                            nc.tensor.matmul(
                                den[:, i0:], ones_h, etr[:, jt, i0:],
                                start=(jt == 0), stop=(jt == SEQT - 1))
                        adr1 = stats.tile([1, 512], DT_F, name="adr1")
                        nc.vector.reciprocal_approx_fast(out=adr1, in_=den)
                        adrb = stats.tile([64, 512], DT_F, name="adrb")
                        nc.gpsimd.partition_broadcast(adrb, adr1)
                        if po == 0:
                            nc.vector.tensor_mul(
                                of[0:64, dt, tsl], ot[0:64, :], adrb)
                        else:
                            # DVE can't shift partitions: normalize at base 0,
                            # DMA-copy up to partitions 64:128.
                            otmp = sm2.tile([64, 512], DT_H, name="otmp")
                            nc.vector.tensor_mul(otmp, ot[0:64, :], adrb)
                            nc.sync.dma_start(
                                out=of[64:128, dt, tsl], in_=otmp)

            for dt in range(DTILES):
                wt = wsm.tile([P, KTILES, P], DT_H, name="wsm_t")
                nc.sync.dma_start(out=wt, in_=par["wot"][l, dt])
                for b in range(BPC):
                    pq = psm.tile([P, 512], DT_F, name="pmain")
                    for kt in range(KTILES):
                        nc.tensor.matmul(
                            pq, wt[:, kt, :], of[:, kt, bass.ts(b, 512)],
                            start=(kt == 0), stop=(kt == 7))
                    nc.scalar.activation(pq, pq, FX.Identity,
                                         bias=bov_t[:, l, dt:dt + 1])
                    nc.vector.tensor_add(
                        x[:, dt, bass.ts(b, 512)], pq,
                        x[:, dt, bass.ts(b, 512)])

            h2 = big.tile([P, DTILES, TOK], DT_H, name="bigbuf")
            layer_norm_fm(lambda d: x[:, d, :], lambda d: h2[:, d, :],
                          DTILES, DIM, TOK,
                          lambda d: lnfg_t[:, l, d:d + 1],
                          lambda d: lnfb_t[:, l, d:d + 1])

            for th in range(2):
                tsl = bass.ts(th, 512)
                with tc.tile_pool(name=f"h1p{l}_{th}", bufs=1) as h1p:
                    h1r = h1p.tile([P, MTILES, 512], DT_H, name="h1r")
                    for mt in range(MTILES):
                        wt = wsm.tile([P, KTILES, P], DT_H, name="wsm_t")
                        nc.sync.dma_start(out=wt, in_=par["w1t"][l, mt])
                        pq = psm.tile([P, 512], DT_F, name="pmain")
                        for kt in range(KTILES):
                            nc.tensor.matmul(pq, wt[:, kt, :], h2[:, kt, tsl],
                                             start=(kt == 0), stop=(kt == 7))
                        nc.scalar.activation(
                            h1r[:, mt, :], pq, FX.Gelu,
                            bias=b1v_t[:, l, mt:mt + 1], scale=1.0)
                    for dt in range(DTILES):
                        pq = psm.tile([P, 512], DT_F, name="pmain")
                        for kh in range(2):
                            wt = wsm.tile([P, 16, P], DT_H, name="wsm_t")
                            nc.sync.dma_start(
                                out=wt, in_=par["w2t"][l, dt, kh])
                            for k2 in range(16):
                                kt = kh * 16 + k2
                                nc.tensor.matmul(
                                    pq, wt[:, k2, :], h1r[:, kt, :],
                                    start=(kt == 0), stop=(kt == 31))
                        nc.scalar.activation(pq, pq, FX.Identity,
                                             bias=b2v_t[:, l, dt:dt + 1])
                        nc.vector.tensor_add(
                            x[:, dt, tsl], pq, x[:, dt, tsl])

        # =================== head ===================
        ho = big.tile([P, DTILES, TOK], DT_R, name="bigbuf")
        layer_norm_fm(lambda d: x[:, d, :], lambda d: ho[:, d, :],
                      DTILES, DIM, TOK,
                      lambda d: lnog_t[:, d:d + 1],
                      lambda d: lnob_t[:, d:d + 1])
        wp3 = par["wproj"].rearrange("(ko p) m -> p ko m", p=P)
        wt = wsm.tile([P, KTILES, NCLS], DT_R, name="wsm_t")
        nc.sync.dma_start(out=wt, in_=wp3)
        out_sb = one1.tile([NCLS, TOK], DT_F, name="out_sb")
        for th in range(2):
            pq = pso.tile([P, 512], DT_F, name="ot")
            for kt in range(KTILES):
                nc.tensor.matmul(pq[0:NCLS, :], wt[:, kt, :],
                                 ho[:, kt, bass.ts(th, 512)],
                                 start=(kt == 0), stop=(kt == 7))
            nc.scalar.activation(out_sb[:, bass.ts(th, 512)], pq[0:NCLS, :],
                                 FX.Identity, bias=bproj_t)
        nc.sync.dma_start(out=par["out"][:, :], in_=out_sb)


# ============================================================
# host side
# ============================================================

_NC_CACHE = None


def _bf16(a):
    import ml_dtypes
    return np.ascontiguousarray(a.astype(ml_dtypes.bfloat16))


def _pack_qk(w):      # [D, DIM, 3072] -> [D, 16, P, 8, P]
    v = w[:, :, :2048].reshape(DEPTH, 8, P, 16, P).transpose(0, 3, 2, 1, 4)
    return _bf16(v)


def _pack_v(w):       # -> [D, 2, 8, P, 512]
    v = w[:, :, 2048:].reshape(DEPTH, 8, P, 2, 512).transpose(0, 3, 1, 2, 4)
    return _bf16(v)


def _pack_kxm(w):     # [D, K, M] -> [D, M//P, P, K//P, P]
    D, K, M = w.shape
    v = w.reshape(D, K // P, P, M // P, P).transpose(0, 3, 2, 1, 4)
    return _bf16(v)


def _pack_w2(w):      # [D, 4096, 1024] -> [D, 8, 2, P, 16, P]
    v = w.reshape(DEPTH, 2, 16, P, 8, P).transpose(0, 4, 1, 3, 2, 5)
    return _bf16(v)


def _host_band():
    tt = np.arange(KSIZE, dtype=np.float64)
    kern = np.exp(-0.5 * ((tt - (KSIZE - 1) / 2.0) / SIGMA) ** 2)
    kern = (kern / kern.sum()).astype(np.float32)
    pad_l = (KSIZE - 1) // 2  # 9
    nt = T // P
    bandc = np.zeros((nt, 3, P, P), dtype=np.float32)
    for ct in range(nt):
        for s in range(3):
            kt = ct - 1 + s
            if not (0 <= kt < nt):
                continue
            rows = np.arange(kt * P, (kt + 1) * P)
            cols = np.arange(ct * P, (ct + 1) * P)
            d = rows[:, None] - cols[None, :] + pad_l
            m = (d >= 0) & (d < KSIZE)
            blk = np.zeros((P, P), np.float32)
            blk[m] = kern[d[m]]
            bandc[ct, s] = blk
    return bandc


def _host_etab(rel_tab):
    i = np.arange(SEQ)
    j = i[:, None]
    rel = np.clip(i[None, :] - j, -(MAXREL - 1), MAXREL - 1) + MAXREL - 1
    et = np.zeros((DEPTH, SEQ, SEQ), dtype=np.float32)
    for l in range(DEPTH):
        e = np.exp(rel_tab[l][rel])
        e[j > i[None, :]] = 0.0
        et[l] = e
    return et.reshape(DEPTH, SEQT, P, SEQ)


def kernel(**inputs):
    global _NC_CACHE
    if _NC_CACHE is None:
        _NC_CACHE = build_nc()
    nc = _NC_CACHE

    f32 = lambda a: np.ascontiguousarray(np.asarray(a, dtype=np.float32))
    import ml_dtypes
    shared = {
        "band": _host_band(),
        "etab": _host_etab(f32(inputs["rel_tab"])),
        "wpe": f32(inputs["W_pe"]),
        "wqk_t": _pack_qk(f32(inputs["Wqkv"])),
        "wv_t": _pack_v(f32(inputs["Wqkv"])),
        "wot": _pack_kxm(f32(inputs["Wo"])),
        "w1t": _pack_kxm(f32(inputs["W1"])),
        "w2t": _pack_w2(f32(inputs["W2"])),
        "wproj": f32(inputs["Wproj"]),
        "lnp1g": f32(inputs["ln_p1_g"]), "lnp1b": f32(inputs["ln_p1_b"]),
        "bpe": f32(inputs["b_pe"]),
        "lnp2g": f32(inputs["ln_p2_g"]), "lnp2b": f32(inputs["ln_p2_b"]),
        "lnag": f32(inputs["ln_a_g"]), "lnab": f32(inputs["ln_a_b"]),
        "lnfg": f32(inputs["ln_f_g"]), "lnfb": f32(inputs["ln_f_b"]),
        "bov": f32(inputs["bo"]), "b1v": f32(inputs["b1"]),
        "b2v": f32(inputs["b2"]),
        "lnog": f32(inputs["ln_o_g"]), "lnob": f32(inputs["ln_o_b"]),
        "bprojv": f32(inputs["bproj"]),
    }
    xfull = f32(inputs["neuralInput"])
    in_maps = []
    for c in range(NCORES):
        m = dict(shared)
        m["xin"] = np.ascontiguousarray(xfull[c * BPC:(c + 1) * BPC])
        in_maps.append(m)

    import os
    trace = bool(os.environ.get("BIT_TRACE"))
    res = run_bass_kernel_spmd(nc, in_maps, list(range(NCORES)), trace=trace)
    if trace:
        globals()["LAST_RESULT"] = res
    outs = []
    for c in range(NCORES):
        o = res.results[c]["out"]              # [NCLS, TOK]
        o = o.reshape(NCLS, BPC, SEQ).transpose(1, 2, 0)
        outs.append(o)
    return np.concatenate(outs, axis=0).astype(np.float32)
